# revision 22
# baseline (speedup 1.0000x reference)
"""Trainium2 Bass kernel for nn_BDGKTLayers (GNN message passing).

Host wrapper: shards the 8192-node user/item graphs across 8 NeuronCores
(1024 users + 1024 items per core), stages layout-transformed fp16 inputs,
runs one SPMD Bass/Tile program per core, reassembles full outputs.

Gathers are hoisted to the host as pure row-copies of RAW inputs (no host
FLOPs) by commuting them with the linear projections, which are folded
into weights:
  item attention logits  e[i,l] = Qt1[i]*ud[uidx[i,l]] + Qt3[i]*resp[i,l]
       with ud = uf@Wu.T  ->  (Qt1@Wu)[i] * uf[uidx[i,l]]
  user recurrence per-edge input T1[idx] = (A@Wi)@if[idx] + (B@l1a)@ist[idx]
       + (B@l1b)@skl[idx] (+ folded bias), staged transposed.
The big per-edge "key"/"V" matmuls of the reference are eliminated
algebraically (softmax shift invariance; V-sum linearity).  All model
FLOPs (projections, attention reduce, softmax, 32-step gated recurrence)
run on device; fp16 operands, fp32 psum accumulation, fp16 state
(validated ~8e-4 rel err vs fp32 reference).
"""

import os
import sys

sys.path.insert(0, "/opt/trn_rl_repo")

from contextlib import ExitStack

import numpy as np

import concourse.bass as bass
import concourse.tile as tile
from concourse import bacc, mybir
from concourse import bass_utils
from concourse.masks import make_identity

F16 = mybir.dt.float16
F32 = mybir.dt.float32
I16 = mybir.dt.int16

NN = 8192          # nodes (users == items)
D = 128
L = 32             # mailbox length
NCORES = 8
NPC = NN // NCORES  # nodes per core = 1024
NT = NPC // 128     # 128-row item tiles per core = 8
UCH = 256           # user chunk (matmul free dim)
NCH = NPC // UCH    # chunks per core = 2
SQD = float(np.sqrt(D))

_CACHE = {}


def _bcast(ap2d, n, pos):
    """Insert a 0-stride dim of size n into a 2D AP at free position pos."""
    new = [list(ap2d.ap[0])]
    free = [list(p) for p in ap2d.ap[1:]]
    free.insert(pos, [0, n])
    return bass.AP(tensor=ap2d.tensor, offset=ap2d.offset, ap=new + free)


def _emit(ctx: ExitStack, tc: tile.TileContext, t):
    nc = tc.nc
    ctx.enter_context(nc.allow_low_precision("fp16 logit/rbar accumulation, validated"))
    ACT = mybir.ActivationFunctionType

    consts = ctx.enter_context(tc.tile_pool(name="consts", bufs=1))

    def load_const(name, shape, dtype):
        s = consts.tile(shape, dtype, tag=name, name=name)
        nc.sync.dma_start(out=s, in_=t[name].ap())
        return s

    w16 = {}
    for n_ in ("wl1a", "wl1b", "wl3", "wq1", "wq3", "wl4a", "wl4b",
               "wLo2", "wfo1", "wC", "wfo2", "wLo1", "wLq",
               "wWuraw", "wAW", "wW1", "wW2"):
        w16[n_] = load_const(n_, [D, D], F16)
    bias = {}
    for n_ in ("bq1", "bLo", "bLq", "bfo", "bl1", "bl3", "bl4", "kini"):
        bias[n_] = load_const(n_, [D, 1], F32)
    ident = consts.tile([D, D], F16, tag="ident")
    make_identity(nc, ident)
    ident32 = consts.tile([D, D], F32, tag="ident32")
    make_identity(nc, ident32)

    keep = ctx.enter_context(tc.tile_pool(name="keep", bufs=1))
    ia_own = keep.tile([128, NPC], F16, tag="ia_own")
    qt1n = keep.tile([128, NT, D], F16, tag="qt1n")   # (Qt1@Wu), natural
    qt3n = keep.tile([128, NT, D], F16, tag="qt3n")
    oi_sb = keep.tile([128, NPC], F32, tag="oi_sb")
    rball = keep.tile([128, NT, D], F16, tag="rball")

    # ---------------- Phase A: own-item tables ----------------
    with tc.tile_pool(name="phA", bufs=1) as pha, \
         tc.tile_pool(name="phA_ps", bufs=2, space="PSUM") as phaps, \
         tc.tile_pool(name="phA_tps", bufs=2, space="PSUM") as phatps, \
         tc.tile_pool(name="phA_s", bufs=3) as phas:
        qt1t = pha.tile([128, NPC], F16, tag="qt1t")
        qt3t = pha.tile([128, NPC], F16, tag="qt3t")
        iown = pha.tile([128, NPC], F16, tag="iown")
        sown = pha.tile([128, NPC], F16, tag="sown")
        nc.sync.dma_start(out=iown, in_=t["ist_own"].ap())
        nc.sync.dma_start(out=sown, in_=t["skl_own"].ap())
        for ch in range(NPC // 512):
            sl = slice(ch * 512, (ch + 1) * 512)
            ps = phaps.tile([128, 512], F32, tag="ps")
            nc.tensor.matmul(ps, w16["wl1a"], iown[:, sl], start=True, stop=False)
            nc.tensor.matmul(ps, w16["wl1b"], sown[:, sl], start=False, stop=True)
            nc.scalar.activation(ia_own[:, sl], ps, ACT.Identity, bias=bias["bl1"])
            ps2 = phaps.tile([128, 512], F32, tag="ps")
            qT = phas.tile([128, 512], F16, tag="qT")
            nc.tensor.matmul(ps2, w16["wl3"], ia_own[:, sl])
            nc.scalar.activation(qT, ps2, ACT.Identity, bias=bias["bl3"])
            ps3 = phaps.tile([128, 512], F32, tag="ps")
            qt1 = phas.tile([128, 512], F16, tag="qt1")
            nc.tensor.matmul(ps3, w16["wq1"], qT)
            nc.scalar.activation(qt1, ps3, ACT.Copy)
            ps5 = phaps.tile([128, 512], F32, tag="ps")
            nc.tensor.matmul(ps5, w16["wWuraw"], qt1)   # (Qt1@Wu)^T
            nc.vector.tensor_copy(qt1t[:, sl], ps5)
            ps4 = phaps.tile([128, 512], F32, tag="ps")
            nc.tensor.matmul(ps4, w16["wq3"], qT)
            nc.vector.tensor_copy(qt3t[:, sl], ps4)
        for tt in range(NT):
            pst = phatps.tile([128, 128], F16, tag="qtt")
            nc.tensor.transpose(pst, qt1t[:, tt * 128:(tt + 1) * 128], ident)
            nc.vector.tensor_copy(qt1n[:, tt, :], pst)
            pst2 = phatps.tile([128, 128], F16, tag="qtt")
            nc.tensor.transpose(pst2, qt3t[:, tt * 128:(tt + 1) * 128], ident)
            nc.vector.tensor_copy(qt3n[:, tt, :], pst2)

    # ---------------- Phases C + B interleaved (shared pool) ----------------
    with tc.tile_pool(name="work", bufs=1) as work, \
         tc.tile_pool(name="phC_ps", bufs=2, space="PSUM") as phcps:

        def emit_c_chunk(chi):
            sl = slice(chi * UCH, (chi + 1) * UCH)
            rsb = work.tile([128, L, UCH], F16, tag="rsb", name="rsb", bufs=2)
            nc.sync.dma_start(out=rsb,
                              in_=t["rT"].ap()[:, :, sl].rearrange("t p u -> p t u"))
            ifg = work.tile([128, L, UCH], F16, tag="ifg", name="ifg", bufs=1)
            nc.sync.dma_start(out=ifg, in_=t["ifgT"].ap()[chi])
            istg = work.tile([128, L, UCH], F16, tag="istg", name="istg", bufs=1)
            nc.sync.dma_start(out=istg, in_=t["istgT"].ap()[chi])
            sklg = work.tile([128, L, UCH], F16, tag="sklg", name="sklg", bufs=1)
            nc.sync.dma_start(out=sklg, in_=t["sklgT"].ap()[chi])
            k = work.tile([128, UCH], F16, tag="k", name="k", bufs=4)
            nc.vector.memset(k, 0.0)
            nc.vector.tensor_scalar_add(k, k, bias["kini"])
            for st in range(L):
                psA = phcps.tile([128, UCH], F32, tag="psA")
                nc.tensor.matmul(psA, w16["wAW"], ifg[:, st, :],
                                 start=True, stop=False)
                nc.tensor.matmul(psA, w16["wW1"], istg[:, st, :],
                                 start=False, stop=False)
                nc.tensor.matmul(psA, w16["wW2"], sklg[:, st, :],
                                 start=False, stop=False)
                nc.tensor.matmul(psA, w16["wC"], k, start=False, stop=True)
                q1 = work.tile([128, UCH], F16, tag="q1", name="q1", bufs=3)
                nc.vector.tensor_scalar_add(q1, psA, bias["bq1"])
                psB = phcps.tile([128, UCH], F32, tag="psB")
                nc.tensor.matmul(psB, w16["wLo2"], rsb[:, st, :],
                                 start=True, stop=False)
                nc.tensor.matmul(psB, w16["wLo1"], q1, start=False, stop=True)
                psC = phcps.tile([128, UCH], F32, tag="psC")
                nc.tensor.matmul(psC, w16["wLq"], q1)
                psD = phcps.tile([128, UCH], F32, tag="psD")
                nc.tensor.matmul(psD, w16["wfo1"], rsb[:, st, :],
                                 start=True, stop=False)
                nc.tensor.matmul(psD, w16["wfo2"], k, start=False, stop=True)
                sg = work.tile([128, UCH], F16, tag="sg", name="sg", bufs=3)
                nc.scalar.activation(sg, psB, ACT.Sigmoid, bias=bias["bLo"])
                th = work.tile([128, UCH], F16, tag="th", name="th", bufs=3)
                nc.scalar.activation(th, psC, ACT.Tanh, bias=bias["bLq"])
                ff = work.tile([128, UCH], F16, tag="ff", name="ff", bufs=3)
                nc.scalar.activation(ff, psD, ACT.Sigmoid, bias=bias["bfo"])
                xx = work.tile([128, UCH], F16, tag="xx", name="xx", bufs=3)
                nc.vector.tensor_mul(xx, sg, th)
                d1 = work.tile([128, UCH], F16, tag="d1", name="d1", bufs=3)
                nc.vector.tensor_sub(d1, k, xx)
                d2 = work.tile([128, UCH], F16, tag="d2", name="d2", bufs=3)
                nc.vector.tensor_mul(d2, ff, d1)
                nc.vector.tensor_add(k, d2, xx)
            nc.sync.dma_start(out=t["out_u"].ap()[:, sl], in_=k)

        def emit_b_tile(tt):
            ug = work.tile([128, L, D], F16, tag="ug", name="ug", bufs=2)
            nc.sync.dma_start(out=ug, in_=t["ufg"].ap()[tt])
            rsp = work.tile([128, L, D], F16, tag="rsp", name="rsp", bufs=2)
            nc.sync.dma_start(out=rsp, in_=t["iresp"].ap()[tt])
            p1 = work.tile([128, L, D], F16, tag="p1", name="p1", bufs=3)
            nc.vector.tensor_mul(p1, ug, _bcast(qt1n[:, tt, :], L, 0))
            p2 = work.tile([128, L, D], F16, tag="p1", name="p2", bufs=3)
            nc.vector.tensor_mul(p2, rsp, _bcast(qt3n[:, tt, :], L, 0))
            s1 = work.tile([128, L, D], F16, tag="p1", name="s1", bufs=3)
            nc.vector.tensor_add(s1, p1, p2)
            s2 = work.tile([128, L, D // 2], F16, tag="s2", name="s2", bufs=2)
            nc.vector.tensor_add(s2, s1[:, :, 0:D // 2], s1[:, :, D // 2:D])
            s3 = work.tile([128, L, D // 4], F16, tag="s3", name="s3", bufs=2)
            nc.vector.tensor_add(s3, s2[:, :, 0:D // 4], s2[:, :, D // 4:D // 2])
            e = work.tile([128, L], F16, tag="e", name="e", bufs=2)
            nc.vector.tensor_reduce(e, s3[:], axis=mybir.AxisListType.X,
                                    op=mybir.AluOpType.add)
            m = work.tile([128, 1], F32, tag="m", name="m", bufs=2)
            nc.vector.tensor_reduce(m, e[:], axis=mybir.AxisListType.X,
                                    op=mybir.AluOpType.max)
            nm = work.tile([128, 1], F32, tag="nm", name="nm", bufs=2)
            nc.vector.tensor_scalar_mul(nm, m, -1.0 / SQD)
            al = work.tile([128, L], F32, tag="al", name="al", bufs=2)
            ssum = work.tile([128, 1], F32, tag="ssum", name="ssum", bufs=2)
            nc.scalar.activation(al, e, ACT.Exp, bias=nm, scale=1.0 / SQD,
                                 accum_out=ssum)
            rinv = work.tile([128, 1], F32, tag="rinv", name="rinv", bufs=2)
            nc.vector.reciprocal(rinv, ssum)
            al16 = work.tile([128, L], F16, tag="al16", name="al16", bufs=2)
            nc.vector.tensor_scalar_mul(al16, al, rinv)
            p3 = work.tile([128, L, D], F16, tag="p1", name="p3", bufs=3)
            nc.vector.tensor_mul(p3, rsp, _bcast(al16[:], D, 1))
            p3v = bass.AP(tensor=p3.tensor, offset=p3.offset,
                          ap=[list(p3.ap[0]), [1, D], [D, L]])
            nc.vector.tensor_reduce(rball[:, tt, :], p3v,
                                    axis=mybir.AxisListType.X,
                                    op=mybir.AluOpType.add, opt_input=False)

        for chi in range(NCH):
            emit_c_chunk(chi)
            for tt in range(2 * chi, 2 * chi + 2):
                emit_b_tile(tt)

    # ---------------- Phase B tail: rbar transpose + item output ----------------
    with tc.tile_pool(name="phBt_s", bufs=2) as phbts, \
         tc.tile_pool(name="phBt_ps", bufs=2, space="PSUM") as phbtps:
        for tt in range(NT):
            pst = phbtps.tile([128, 128], F16, tag="rbT")
            nc.tensor.transpose(pst, rball[:, tt, :], ident)
            rbT = phbts.tile([128, 128], F16, tag="rbTs")
            nc.vector.tensor_copy(rbT, pst)
            psI = phbtps.tile([128, 128], F32, tag="psI")
            nc.tensor.matmul(psI, w16["wl4a"], ia_own[:, tt * 128:(tt + 1) * 128],
                             start=True, stop=False)
            nc.tensor.matmul(psI, w16["wl4b"], rbT, start=False, stop=True)
            nc.scalar.activation(oi_sb[:, tt * 128:(tt + 1) * 128], psI,
                                 ACT.Identity, bias=bias["bl4"])
        nc.sync.dma_start(out=t["out_i"].ap(), in_=oi_sb)


def build_program():
    if "nc" in _CACHE:
        return _CACHE["nc"], _CACHE["names"]
    nc = bacc.Bacc("TRN2", target_bir_lowering=False, debug=False,
                   num_devices=NCORES)
    t = {}

    def din(name, shape, dtype):
        t[name] = nc.dram_tensor(name, shape, dtype, kind="ExternalInput")

    for n_ in ("wl1a", "wl1b", "wl3", "wq1", "wq3", "wl4a", "wl4b",
               "wLo2", "wfo1", "wC", "wfo2", "wLo1", "wLq",
               "wWuraw", "wAW", "wW1", "wW2"):
        din(n_, [D, D], F16)
    for n_ in ("bq1", "bLo", "bLq", "bfo", "bl1", "bl3", "bl4", "kini"):
        din(n_, [D, 1], F32)
    din("ist_own", [128, NPC], F16)
    din("skl_own", [128, NPC], F16)
    din("ufg", [NT, 128, L * D], F16)     # user_feat[item mailbox idx], natural
    din("iresp", [NT, 128, L * D], F16)
    din("rT", [L, 128, NPC], F16)         # user_mail_resp, transposed
    din("ifgT", [NCH, 128, L, UCH], F16)  # item_feat[user mailbox idx], transposed
    din("istgT", [NCH, 128, L, UCH], F16)
    din("sklgT", [NCH, 128, L, UCH], F16)
    t["out_u"] = nc.dram_tensor("out_u", [128, NPC], F16, kind="ExternalOutput")
    t["out_i"] = nc.dram_tensor("out_i", [128, NPC], F32, kind="ExternalOutput")

    with tile.TileContext(nc) as tc:
        with ExitStack() as ctx:
            _emit(ctx, tc, t)
    nc.compile()
    _CACHE["nc"] = nc
    _CACHE["names"] = t
    return nc, t


def prepare_inputs(user_feat, item_feat, user_static, item_static, skill,
                   user_mail_item_idx, user_mail_resp, item_mail_user_idx,
                   item_mail_resp, params):
    p = {k: np.asarray(v, dtype=np.float32) for k, v in params.items()}
    f16 = np.float16

    A, B = p["q1_W"][:, :D], p["q1_W"][:, D:2 * D]
    AW = A @ p["Wi"]                      # folded (A @ Wi)
    W1 = B @ p["l1_W"][:, :D]
    W2 = B @ p["l1_W"][:, D:]
    bq1f = p["q1_b"] + B @ p["l1_b"]      # folded bias

    rep = {
        "wl1a": np.ascontiguousarray(p["l1_W"][:, :D].T, dtype=f16),
        "wl1b": np.ascontiguousarray(p["l1_W"][:, D:].T, dtype=f16),
        "wl3": np.ascontiguousarray(p["l3_W"].T, dtype=f16),
        "wq1": np.ascontiguousarray(p["l2_W"][:, :D], dtype=f16),
        "wq3": np.ascontiguousarray(p["l2_W"][:, 2 * D:], dtype=f16),
        "wl4a": np.ascontiguousarray(p["l4_W"][:, :D].T, dtype=f16),
        "wl4b": np.ascontiguousarray(p["l4_W"][:, D:].T, dtype=f16),
        "wLo2": np.ascontiguousarray(p["Lo_W"][:, D:].T, dtype=f16),
        "wfo1": np.ascontiguousarray(p["fo_W"][:, :D].T, dtype=f16),
        "wC": np.ascontiguousarray(p["q1_W"][:, 2 * D:].T, dtype=f16),
        "wfo2": np.ascontiguousarray(p["fo_W"][:, D:].T, dtype=f16),
        "wLo1": np.ascontiguousarray(p["Lo_W"][:, :D].T, dtype=f16),
        "wLq": np.ascontiguousarray(p["Lq_W"].T, dtype=f16),
        "wWuraw": np.ascontiguousarray(p["Wu"], dtype=f16),
        "wAW": np.ascontiguousarray(AW.T, dtype=f16),
        "wW1": np.ascontiguousarray(W1.T, dtype=f16),
        "wW2": np.ascontiguousarray(W2.T, dtype=f16),
        "bq1": np.ascontiguousarray(bq1f[:, None], dtype=np.float32),
        "bLo": np.ascontiguousarray(p["Lo_b"][:, None], dtype=np.float32),
        "bLq": np.ascontiguousarray(p["Lq_b"][:, None], dtype=np.float32),
        "bfo": np.ascontiguousarray(p["fo_b"][:, None], dtype=np.float32),
        "bl1": np.ascontiguousarray(p["l1_b"][:, None], dtype=np.float32),
        "bl3": np.ascontiguousarray(p["l3_b"][:, None], dtype=np.float32),
        "bl4": np.ascontiguousarray(p["l4_b"][:, None], dtype=np.float32),
        "kini": np.ascontiguousarray(p["k_init"][0][:, None], dtype=np.float32),
    }

    uidx_full = np.asarray(user_mail_item_idx).astype(np.int64)
    iidx_full = np.asarray(item_mail_user_idx).astype(np.int64)
    uresp = np.asarray(user_mail_resp, dtype=np.float32)
    iresp = np.asarray(item_mail_resp, dtype=np.float32)
    uf16 = np.asarray(user_feat).astype(f16)
    if16 = np.asarray(item_feat).astype(f16)
    ist16 = np.asarray(item_static).astype(f16)
    skl16 = np.asarray(skill).astype(f16)
    ist_T = np.asarray(item_static).T
    skl_T = np.asarray(skill).T

    def stage_user_gather(tab16, idx):
        # [1024, L, D] -> [NCH, 128(d), L, UCH] (transposed, per chunk)
        g = tab16[idx]                                  # [1024, L, D]
        g = g.reshape(NCH, UCH, L, D).transpose(0, 3, 2, 1)
        return np.ascontiguousarray(g)

    in_maps = []
    for c in range(NCORES):
        s = slice(c * NPC, (c + 1) * NPC)
        ui = uidx_full[s]
        m = {
            "ist_own": np.ascontiguousarray(ist_T[:, s], dtype=f16),
            "skl_own": np.ascontiguousarray(skl_T[:, s], dtype=f16),
            "ufg": np.ascontiguousarray(
                uf16[iidx_full[s]].reshape(NT, 128, L * D)),
            "iresp": np.ascontiguousarray(
                iresp[s].reshape(NT, 128, L * D), dtype=f16),
            "rT": np.ascontiguousarray(uresp[s].transpose(1, 2, 0), dtype=f16),
            "ifgT": stage_user_gather(if16, ui),
            "istgT": stage_user_gather(ist16, ui),
            "sklgT": stage_user_gather(skl16, ui),
        }
        m.update(rep)
        in_maps.append(m)
    return in_maps


def kernel(user_feat, item_feat, user_static, item_static, skill,
           user_mail_item_idx, user_mail_resp, item_mail_user_idx,
           item_mail_resp, params):
    nc, _ = build_program()
    in_maps = prepare_inputs(
        user_feat, item_feat, user_static, item_static, skill,
        user_mail_item_idx, user_mail_resp, item_mail_user_idx,
        item_mail_resp, params)
    trace = os.environ.get("KERNEL_TRACE", "0") == "1"
    res = bass_utils.run_bass_kernel_spmd(
        nc, in_maps, core_ids=list(range(NCORES)), trace=trace)
    _CACHE["last_result"] = res
    user_new = np.concatenate(
        [res.results[c]["out_u"].T for c in range(NCORES)], axis=0)
    item_new = np.concatenate(
        [res.results[c]["out_i"].T for c in range(NCORES)], axis=0)
    return (np.ascontiguousarray(user_new, dtype=np.float32),
            np.ascontiguousarray(item_new, dtype=np.float32))


# revision 23
# speedup vs baseline: 1.0160x; 1.0160x over previous
"""Trainium2 Bass kernel for nn_BDGKTLayers (GNN message passing).

Host wrapper: shards the 8192-node user/item graphs across 8 NeuronCores
(1024 users + 1024 items per core), stages layout-transformed fp16 inputs,
runs one SPMD Bass/Tile program per core, reassembles full outputs.

Gathers are hoisted to the host as pure row-copies of RAW inputs (no host
FLOPs) by commuting them with the linear projections, which are folded
into weights:
  item attention logits  e[i,l] = Qt1[i]*ud[uidx[i,l]] + Qt3[i]*resp[i,l]
       with ud = uf@Wu.T  ->  (Qt1@Wu)[i] * uf[uidx[i,l]]
  user recurrence per-edge input T1[idx] = (A@Wi)@if[idx] + (B@l1a)@ist[idx]
       + (B@l1b)@skl[idx] (+ folded bias), staged transposed.
The big per-edge "key"/"V" matmuls of the reference are eliminated
algebraically (softmax shift invariance; V-sum linearity).  All model
FLOPs (projections, attention reduce, softmax, 32-step gated recurrence)
run on device; fp16 operands, fp32 psum accumulation, fp16 state
(validated ~8e-4 rel err vs fp32 reference).
"""

import os
import sys

sys.path.insert(0, "/opt/trn_rl_repo")

from contextlib import ExitStack

import numpy as np

import concourse.bass as bass
import concourse.tile as tile
from concourse import bacc, mybir
from concourse import bass_utils
from concourse.masks import make_identity

F16 = mybir.dt.float16
F32 = mybir.dt.float32
I16 = mybir.dt.int16

NN = 8192          # nodes (users == items)
D = 128
L = 32             # mailbox length
NCORES = 8
NPC = NN // NCORES  # nodes per core = 1024
NT = NPC // 128     # 128-row item tiles per core = 8
UCH = 256           # user chunk (matmul free dim)
NCH = NPC // UCH    # chunks per core = 2
SQD = float(np.sqrt(D))

_CACHE = {}


def _bcast(ap2d, n, pos):
    """Insert a 0-stride dim of size n into a 2D AP at free position pos."""
    new = [list(ap2d.ap[0])]
    free = [list(p) for p in ap2d.ap[1:]]
    free.insert(pos, [0, n])
    return bass.AP(tensor=ap2d.tensor, offset=ap2d.offset, ap=new + free)


def _emit(ctx: ExitStack, tc: tile.TileContext, t):
    nc = tc.nc
    ctx.enter_context(nc.allow_low_precision("fp16 logit/rbar accumulation, validated"))
    ACT = mybir.ActivationFunctionType

    consts = ctx.enter_context(tc.tile_pool(name="consts", bufs=1))

    def load_const(name, shape, dtype):
        s = consts.tile(shape, dtype, tag=name, name=name)
        nc.sync.dma_start(out=s, in_=t[name].ap())
        return s

    w16 = {}
    for n_ in ("wl1a", "wl1b", "wl3", "wq1", "wq3", "wl4a", "wl4b",
               "wLo2", "wfo1", "wC", "wfo2", "wLo1", "wLq",
               "wWuraw", "wAW", "wW1", "wW2"):
        w16[n_] = load_const(n_, [D, D], F16)
    bias = {}
    for n_ in ("bq1", "bLo", "bLq", "bfo", "bl1", "bl3", "bl4", "kini"):
        bias[n_] = load_const(n_, [D, 1], F32)
    ident = consts.tile([D, D], F16, tag="ident")
    make_identity(nc, ident)
    ident32 = consts.tile([D, D], F32, tag="ident32")
    make_identity(nc, ident32)

    keep = ctx.enter_context(tc.tile_pool(name="keep", bufs=1))
    ia_own = keep.tile([128, NPC], F16, tag="ia_own")
    qt1n = keep.tile([128, NT, D], F16, tag="qt1n")   # (Qt1@Wu), natural
    qt3n = keep.tile([128, NT, D], F16, tag="qt3n")
    oi_sb = keep.tile([128, NPC], F32, tag="oi_sb")
    rball = keep.tile([128, NT, D], F16, tag="rball")

    # ---------------- Phase A: own-item tables ----------------
    with tc.tile_pool(name="phA", bufs=1) as pha, \
         tc.tile_pool(name="phA_ps", bufs=2, space="PSUM") as phaps, \
         tc.tile_pool(name="phA_tps", bufs=2, space="PSUM") as phatps, \
         tc.tile_pool(name="phA_s", bufs=3) as phas:
        qt1t = pha.tile([128, NPC], F16, tag="qt1t")
        qt3t = pha.tile([128, NPC], F16, tag="qt3t")
        iown = pha.tile([128, NPC], F16, tag="iown")
        sown = pha.tile([128, NPC], F16, tag="sown")
        nc.sync.dma_start(out=iown, in_=t["ist_own"].ap())
        nc.sync.dma_start(out=sown, in_=t["skl_own"].ap())
        for ch in range(NPC // 512):
            sl = slice(ch * 512, (ch + 1) * 512)
            ps = phaps.tile([128, 512], F32, tag="ps")
            nc.tensor.matmul(ps, w16["wl1a"], iown[:, sl], start=True, stop=False)
            nc.tensor.matmul(ps, w16["wl1b"], sown[:, sl], start=False, stop=True)
            nc.scalar.activation(ia_own[:, sl], ps, ACT.Identity, bias=bias["bl1"])
            ps2 = phaps.tile([128, 512], F32, tag="ps")
            qT = phas.tile([128, 512], F16, tag="qT")
            nc.tensor.matmul(ps2, w16["wl3"], ia_own[:, sl])
            nc.scalar.activation(qT, ps2, ACT.Identity, bias=bias["bl3"])
            ps3 = phaps.tile([128, 512], F32, tag="ps")
            qt1 = phas.tile([128, 512], F16, tag="qt1")
            nc.tensor.matmul(ps3, w16["wq1"], qT)
            nc.scalar.activation(qt1, ps3, ACT.Copy)
            ps5 = phaps.tile([128, 512], F32, tag="ps")
            nc.tensor.matmul(ps5, w16["wWuraw"], qt1)   # (Qt1@Wu)^T
            nc.vector.tensor_copy(qt1t[:, sl], ps5)
            ps4 = phaps.tile([128, 512], F32, tag="ps")
            nc.tensor.matmul(ps4, w16["wq3"], qT)
            nc.vector.tensor_copy(qt3t[:, sl], ps4)
        for tt in range(NT):
            pst = phatps.tile([128, 128], F16, tag="qtt")
            nc.tensor.transpose(pst, qt1t[:, tt * 128:(tt + 1) * 128], ident)
            nc.vector.tensor_copy(qt1n[:, tt, :], pst)
            pst2 = phatps.tile([128, 128], F16, tag="qtt")
            nc.tensor.transpose(pst2, qt3t[:, tt * 128:(tt + 1) * 128], ident)
            nc.vector.tensor_copy(qt3n[:, tt, :], pst2)

    # ---------------- Phases C + B interleaved (shared pool) ----------------
    with tc.tile_pool(name="work", bufs=1) as work, \
         tc.tile_pool(name="phC_ps", bufs=2, space="PSUM") as phcps:

        def emit_c_chunk(chi):
            sl = slice(chi * UCH, (chi + 1) * UCH)
            rsb = work.tile([128, L, UCH], F16, tag="rsb", name="rsb", bufs=2)
            nc.sync.dma_start(out=rsb,
                              in_=t["rT"].ap()[:, :, sl].rearrange("t p u -> p t u"))
            ifg = work.tile([128, L, UCH], F16, tag="ifg", name="ifg", bufs=1)
            nc.sync.dma_start(out=ifg, in_=t["ifgT"].ap()[chi])
            istg = work.tile([128, L, UCH], F16, tag="istg", name="istg", bufs=1)
            nc.sync.dma_start(out=istg, in_=t["istgT"].ap()[chi])
            sklg = work.tile([128, L, UCH], F16, tag="sklg", name="sklg", bufs=1)
            nc.sync.dma_start(out=sklg, in_=t["sklgT"].ap()[chi])
            k = work.tile([128, UCH], F16, tag="k", name="k", bufs=4)
            nc.vector.memset(k, 0.0)
            nc.vector.tensor_scalar_add(k, k, bias["kini"])
            for st in range(L):
                psA = phcps.tile([128, UCH], F32, tag="psA")
                nc.tensor.matmul(psA, w16["wAW"], ifg[:, st, :],
                                 start=True, stop=False)
                nc.tensor.matmul(psA, w16["wW1"], istg[:, st, :],
                                 start=False, stop=False)
                nc.tensor.matmul(psA, w16["wW2"], sklg[:, st, :],
                                 start=False, stop=False)
                nc.tensor.matmul(psA, w16["wC"], k, start=False, stop=True)
                q1 = work.tile([128, UCH], F16, tag="q1", name="q1", bufs=3)
                nc.scalar.activation(q1, psA, ACT.Identity, bias=bias["bq1"])
                psB = phcps.tile([128, UCH], F32, tag="psB")
                nc.tensor.matmul(psB, w16["wLo2"], rsb[:, st, :],
                                 start=True, stop=False)
                nc.tensor.matmul(psB, w16["wLo1"], q1, start=False, stop=True)
                psC = phcps.tile([128, UCH], F32, tag="psC")
                nc.tensor.matmul(psC, w16["wLq"], q1)
                psD = phcps.tile([128, UCH], F32, tag="psD")
                nc.tensor.matmul(psD, w16["wfo1"], rsb[:, st, :],
                                 start=True, stop=False)
                nc.tensor.matmul(psD, w16["wfo2"], k, start=False, stop=True)
                sg = work.tile([128, UCH], F16, tag="sg", name="sg", bufs=3)
                nc.scalar.activation(sg, psB, ACT.Sigmoid, bias=bias["bLo"])
                th = work.tile([128, UCH], F16, tag="th", name="th", bufs=3)
                nc.scalar.activation(th, psC, ACT.Tanh, bias=bias["bLq"])
                ff = work.tile([128, UCH], F16, tag="ff", name="ff", bufs=3)
                nc.scalar.activation(ff, psD, ACT.Sigmoid, bias=bias["bfo"])
                xx = work.tile([128, UCH], F16, tag="xx", name="xx", bufs=3)
                nc.vector.tensor_mul(xx, sg, th)
                d1 = work.tile([128, UCH], F16, tag="d1", name="d1", bufs=3)
                nc.vector.tensor_sub(d1, k, xx)
                d2 = work.tile([128, UCH], F16, tag="d2", name="d2", bufs=3)
                nc.vector.tensor_mul(d2, ff, d1)
                nc.vector.tensor_add(k, d2, xx)
            nc.sync.dma_start(out=t["out_u"].ap()[:, sl], in_=k)

        def emit_b_tile(tt):
            ug = work.tile([128, L, D], F16, tag="ug", name="ug", bufs=2)
            nc.sync.dma_start(out=ug, in_=t["ufg"].ap()[tt])
            rsp = work.tile([128, L, D], F16, tag="rsp", name="rsp", bufs=2)
            nc.sync.dma_start(out=rsp, in_=t["iresp"].ap()[tt])
            p1 = work.tile([128, L, D], F16, tag="p1", name="p1", bufs=3)
            nc.vector.tensor_mul(p1, ug, _bcast(qt1n[:, tt, :], L, 0))
            p2 = work.tile([128, L, D], F16, tag="p1", name="p2", bufs=3)
            nc.vector.tensor_mul(p2, rsp, _bcast(qt3n[:, tt, :], L, 0))
            s1 = work.tile([128, L, D], F16, tag="p1", name="s1", bufs=3)
            nc.vector.tensor_add(s1, p1, p2)
            s2 = work.tile([128, L, D // 2], F16, tag="s2", name="s2", bufs=2)
            nc.vector.tensor_add(s2, s1[:, :, 0:D // 2], s1[:, :, D // 2:D])
            s3 = work.tile([128, L, D // 4], F16, tag="s3", name="s3", bufs=2)
            nc.vector.tensor_add(s3, s2[:, :, 0:D // 4], s2[:, :, D // 4:D // 2])
            e = work.tile([128, L], F16, tag="e", name="e", bufs=2)
            nc.vector.tensor_reduce(e, s3[:], axis=mybir.AxisListType.X,
                                    op=mybir.AluOpType.add)
            m = work.tile([128, 1], F32, tag="m", name="m", bufs=2)
            nc.vector.tensor_reduce(m, e[:], axis=mybir.AxisListType.X,
                                    op=mybir.AluOpType.max)
            nm = work.tile([128, 1], F32, tag="nm", name="nm", bufs=2)
            nc.vector.tensor_scalar_mul(nm, m, -1.0 / SQD)
            al = work.tile([128, L], F32, tag="al", name="al", bufs=2)
            ssum = work.tile([128, 1], F32, tag="ssum", name="ssum", bufs=2)
            nc.scalar.activation(al, e, ACT.Exp, bias=nm, scale=1.0 / SQD,
                                 accum_out=ssum)
            rinv = work.tile([128, 1], F32, tag="rinv", name="rinv", bufs=2)
            nc.vector.reciprocal(rinv, ssum)
            al16 = work.tile([128, L], F16, tag="al16", name="al16", bufs=2)
            nc.vector.tensor_scalar_mul(al16, al, rinv)
            p3 = work.tile([128, L, D], F16, tag="p1", name="p3", bufs=3)
            nc.vector.tensor_mul(p3, rsp, _bcast(al16[:], D, 1))
            p3v = bass.AP(tensor=p3.tensor, offset=p3.offset,
                          ap=[list(p3.ap[0]), [1, D], [D, L]])
            nc.vector.tensor_reduce(rball[:, tt, :], p3v,
                                    axis=mybir.AxisListType.X,
                                    op=mybir.AluOpType.add, opt_input=False)

        for chi in range(NCH):
            emit_c_chunk(chi)
            for tt in range(2 * chi, 2 * chi + 2):
                emit_b_tile(tt)

    # ---------------- Phase B tail: rbar transpose + item output ----------------
    with tc.tile_pool(name="phBt_s", bufs=2) as phbts, \
         tc.tile_pool(name="phBt_ps", bufs=2, space="PSUM") as phbtps:
        for tt in range(NT):
            pst = phbtps.tile([128, 128], F16, tag="rbT")
            nc.tensor.transpose(pst, rball[:, tt, :], ident)
            rbT = phbts.tile([128, 128], F16, tag="rbTs")
            nc.vector.tensor_copy(rbT, pst)
            psI = phbtps.tile([128, 128], F32, tag="psI")
            nc.tensor.matmul(psI, w16["wl4a"], ia_own[:, tt * 128:(tt + 1) * 128],
                             start=True, stop=False)
            nc.tensor.matmul(psI, w16["wl4b"], rbT, start=False, stop=True)
            nc.scalar.activation(oi_sb[:, tt * 128:(tt + 1) * 128], psI,
                                 ACT.Identity, bias=bias["bl4"])
        nc.sync.dma_start(out=t["out_i"].ap(), in_=oi_sb)


def build_program():
    if "nc" in _CACHE:
        return _CACHE["nc"], _CACHE["names"]
    nc = bacc.Bacc("TRN2", target_bir_lowering=False, debug=False,
                   num_devices=NCORES)
    t = {}

    def din(name, shape, dtype):
        t[name] = nc.dram_tensor(name, shape, dtype, kind="ExternalInput")

    for n_ in ("wl1a", "wl1b", "wl3", "wq1", "wq3", "wl4a", "wl4b",
               "wLo2", "wfo1", "wC", "wfo2", "wLo1", "wLq",
               "wWuraw", "wAW", "wW1", "wW2"):
        din(n_, [D, D], F16)
    for n_ in ("bq1", "bLo", "bLq", "bfo", "bl1", "bl3", "bl4", "kini"):
        din(n_, [D, 1], F32)
    din("ist_own", [128, NPC], F16)
    din("skl_own", [128, NPC], F16)
    din("ufg", [NT, 128, L * D], F16)     # user_feat[item mailbox idx], natural
    din("iresp", [NT, 128, L * D], F16)
    din("rT", [L, 128, NPC], F16)         # user_mail_resp, transposed
    din("ifgT", [NCH, 128, L, UCH], F16)  # item_feat[user mailbox idx], transposed
    din("istgT", [NCH, 128, L, UCH], F16)
    din("sklgT", [NCH, 128, L, UCH], F16)
    t["out_u"] = nc.dram_tensor("out_u", [128, NPC], F16, kind="ExternalOutput")
    t["out_i"] = nc.dram_tensor("out_i", [128, NPC], F32, kind="ExternalOutput")

    with tile.TileContext(nc) as tc:
        with ExitStack() as ctx:
            _emit(ctx, tc, t)
    nc.compile()
    _CACHE["nc"] = nc
    _CACHE["names"] = t
    return nc, t


def prepare_inputs(user_feat, item_feat, user_static, item_static, skill,
                   user_mail_item_idx, user_mail_resp, item_mail_user_idx,
                   item_mail_resp, params):
    p = {k: np.asarray(v, dtype=np.float32) for k, v in params.items()}
    f16 = np.float16

    A, B = p["q1_W"][:, :D], p["q1_W"][:, D:2 * D]
    AW = A @ p["Wi"]                      # folded (A @ Wi)
    W1 = B @ p["l1_W"][:, :D]
    W2 = B @ p["l1_W"][:, D:]
    bq1f = p["q1_b"] + B @ p["l1_b"]      # folded bias

    rep = {
        "wl1a": np.ascontiguousarray(p["l1_W"][:, :D].T, dtype=f16),
        "wl1b": np.ascontiguousarray(p["l1_W"][:, D:].T, dtype=f16),
        "wl3": np.ascontiguousarray(p["l3_W"].T, dtype=f16),
        "wq1": np.ascontiguousarray(p["l2_W"][:, :D], dtype=f16),
        "wq3": np.ascontiguousarray(p["l2_W"][:, 2 * D:], dtype=f16),
        "wl4a": np.ascontiguousarray(p["l4_W"][:, :D].T, dtype=f16),
        "wl4b": np.ascontiguousarray(p["l4_W"][:, D:].T, dtype=f16),
        "wLo2": np.ascontiguousarray(p["Lo_W"][:, D:].T, dtype=f16),
        "wfo1": np.ascontiguousarray(p["fo_W"][:, :D].T, dtype=f16),
        "wC": np.ascontiguousarray(p["q1_W"][:, 2 * D:].T, dtype=f16),
        "wfo2": np.ascontiguousarray(p["fo_W"][:, D:].T, dtype=f16),
        "wLo1": np.ascontiguousarray(p["Lo_W"][:, :D].T, dtype=f16),
        "wLq": np.ascontiguousarray(p["Lq_W"].T, dtype=f16),
        "wWuraw": np.ascontiguousarray(p["Wu"], dtype=f16),
        "wAW": np.ascontiguousarray(AW.T, dtype=f16),
        "wW1": np.ascontiguousarray(W1.T, dtype=f16),
        "wW2": np.ascontiguousarray(W2.T, dtype=f16),
        "bq1": np.ascontiguousarray(bq1f[:, None], dtype=np.float32),
        "bLo": np.ascontiguousarray(p["Lo_b"][:, None], dtype=np.float32),
        "bLq": np.ascontiguousarray(p["Lq_b"][:, None], dtype=np.float32),
        "bfo": np.ascontiguousarray(p["fo_b"][:, None], dtype=np.float32),
        "bl1": np.ascontiguousarray(p["l1_b"][:, None], dtype=np.float32),
        "bl3": np.ascontiguousarray(p["l3_b"][:, None], dtype=np.float32),
        "bl4": np.ascontiguousarray(p["l4_b"][:, None], dtype=np.float32),
        "kini": np.ascontiguousarray(p["k_init"][0][:, None], dtype=np.float32),
    }

    uidx_full = np.asarray(user_mail_item_idx).astype(np.int64)
    iidx_full = np.asarray(item_mail_user_idx).astype(np.int64)
    uresp = np.asarray(user_mail_resp, dtype=np.float32)
    iresp = np.asarray(item_mail_resp, dtype=np.float32)
    uf16 = np.asarray(user_feat).astype(f16)
    if16 = np.asarray(item_feat).astype(f16)
    ist16 = np.asarray(item_static).astype(f16)
    skl16 = np.asarray(skill).astype(f16)
    ist_T = np.asarray(item_static).T
    skl_T = np.asarray(skill).T

    def stage_user_gather(tab16, idx):
        # [1024, L, D] -> [NCH, 128(d), L, UCH] (transposed, per chunk)
        g = tab16[idx]                                  # [1024, L, D]
        g = g.reshape(NCH, UCH, L, D).transpose(0, 3, 2, 1)
        return np.ascontiguousarray(g)

    in_maps = []
    for c in range(NCORES):
        s = slice(c * NPC, (c + 1) * NPC)
        ui = uidx_full[s]
        m = {
            "ist_own": np.ascontiguousarray(ist_T[:, s], dtype=f16),
            "skl_own": np.ascontiguousarray(skl_T[:, s], dtype=f16),
            "ufg": np.ascontiguousarray(
                uf16[iidx_full[s]].reshape(NT, 128, L * D)),
            "iresp": np.ascontiguousarray(
                iresp[s].reshape(NT, 128, L * D), dtype=f16),
            "rT": np.ascontiguousarray(uresp[s].transpose(1, 2, 0), dtype=f16),
            "ifgT": stage_user_gather(if16, ui),
            "istgT": stage_user_gather(ist16, ui),
            "sklgT": stage_user_gather(skl16, ui),
        }
        m.update(rep)
        in_maps.append(m)
    return in_maps


def kernel(user_feat, item_feat, user_static, item_static, skill,
           user_mail_item_idx, user_mail_resp, item_mail_user_idx,
           item_mail_resp, params):
    nc, _ = build_program()
    in_maps = prepare_inputs(
        user_feat, item_feat, user_static, item_static, skill,
        user_mail_item_idx, user_mail_resp, item_mail_user_idx,
        item_mail_resp, params)
    trace = os.environ.get("KERNEL_TRACE", "0") == "1"
    res = bass_utils.run_bass_kernel_spmd(
        nc, in_maps, core_ids=list(range(NCORES)), trace=trace)
    _CACHE["last_result"] = res
    user_new = np.concatenate(
        [res.results[c]["out_u"].T for c in range(NCORES)], axis=0)
    item_new = np.concatenate(
        [res.results[c]["out_i"].T for c in range(NCORES)], axis=0)
    return (np.ascontiguousarray(user_new, dtype=np.float32),
            np.ascontiguousarray(item_new, dtype=np.float32))


# revision 25
# speedup vs baseline: 1.3418x; 1.3207x over previous
"""Trainium2 Bass kernel for nn_BDGKTLayers (GNN message passing).

Host wrapper: shards the 8192-node user/item graphs across 8 NeuronCores
(1024 users + 1024 items per core), stages layout-transformed fp16 inputs,
runs one SPMD Bass/Tile program per core, reassembles full outputs.

Gathers are hoisted to the host as pure row-copies of RAW inputs (no host
FLOPs) by commuting them with the linear projections, which are folded
into weights:
  item attention logits  e[i,l] = Qt1[i]*ud[uidx[i,l]] + Qt3[i]*resp[i,l]
       with ud = uf@Wu.T  ->  (Qt1@Wu)[i] * uf[uidx[i,l]]
  user recurrence per-edge input T1[idx] = (A@Wi)@if[idx] + (B@l1a)@ist[idx]
       + (B@l1b)@skl[idx] (+ folded bias), staged transposed.
The big per-edge "key"/"V" matmuls of the reference are eliminated
algebraically (softmax shift invariance; V-sum linearity).  All model
FLOPs (projections, attention reduce, softmax, 32-step gated recurrence)
run on device; fp16 operands, fp32 psum accumulation, fp16 state
(validated ~8e-4 rel err vs fp32 reference).
"""

import os
import sys

sys.path.insert(0, "/opt/trn_rl_repo")

from contextlib import ExitStack

import numpy as np

import concourse.bass as bass
import concourse.tile as tile
from concourse import bacc, mybir
from concourse import bass_utils
from concourse.masks import make_identity

F16 = mybir.dt.float16
F32 = mybir.dt.float32
I16 = mybir.dt.int16

NN = 8192          # nodes (users == items)
D = 128
L = 32             # mailbox length
NCORES = 8
NPC = NN // NCORES  # nodes per core = 1024
NT = NPC // 128     # 128-row item tiles per core = 8
UCH = 512           # user chunk (matmul free dim)
NCH = NPC // UCH    # chunks per core = 2
SQD = float(np.sqrt(D))

_CACHE = {}


def _bcast(ap2d, n, pos):
    """Insert a 0-stride dim of size n into a 2D AP at free position pos."""
    new = [list(ap2d.ap[0])]
    free = [list(p) for p in ap2d.ap[1:]]
    free.insert(pos, [0, n])
    return bass.AP(tensor=ap2d.tensor, offset=ap2d.offset, ap=new + free)


def _emit(ctx: ExitStack, tc: tile.TileContext, t):
    nc = tc.nc
    ctx.enter_context(nc.allow_low_precision("fp16 logit/rbar accumulation, validated"))
    ACT = mybir.ActivationFunctionType

    consts = ctx.enter_context(tc.tile_pool(name="consts", bufs=1))

    def load_const(name, shape, dtype):
        s = consts.tile(shape, dtype, tag=name, name=name)
        nc.sync.dma_start(out=s, in_=t[name].ap())
        return s

    w16 = {}
    for n_ in ("wl1a", "wl1b", "wl3", "wq1", "wq3", "wl4a", "wl4b",
               "wLo2", "wfo1", "wC", "wfo2", "wLo1", "wLq",
               "wWuraw", "wAW", "wW1", "wW2"):
        w16[n_] = load_const(n_, [D, D], F16)
    bias = {}
    for n_ in ("bq1", "bLo", "bLq", "bfo", "bl1", "bl3", "bl4", "kini"):
        bias[n_] = load_const(n_, [D, 1], F32)
    ident = consts.tile([D, D], F16, tag="ident")
    make_identity(nc, ident)
    ident32 = consts.tile([D, D], F32, tag="ident32")
    make_identity(nc, ident32)

    keep = ctx.enter_context(tc.tile_pool(name="keep", bufs=1))
    ia_own = keep.tile([128, NPC], F16, tag="ia_own")
    qt1n = keep.tile([128, NT, D], F16, tag="qt1n")   # (Qt1@Wu), natural
    qt3n = keep.tile([128, NT, D], F16, tag="qt3n")
    oi_sb = keep.tile([128, NPC], F32, tag="oi_sb")
    rball = keep.tile([128, NT, D], F16, tag="rball")

    # ---------------- Phase A: own-item tables ----------------
    with tc.tile_pool(name="phA", bufs=1) as pha, \
         tc.tile_pool(name="phA_ps", bufs=2, space="PSUM") as phaps, \
         tc.tile_pool(name="phA_tps", bufs=2, space="PSUM") as phatps, \
         tc.tile_pool(name="phA_s", bufs=3) as phas:
        qt1t = pha.tile([128, NPC], F16, tag="qt1t")
        qt3t = pha.tile([128, NPC], F16, tag="qt3t")
        iown = pha.tile([128, NPC], F16, tag="iown")
        sown = pha.tile([128, NPC], F16, tag="sown")
        nc.sync.dma_start(out=iown, in_=t["ist_own"].ap())
        nc.sync.dma_start(out=sown, in_=t["skl_own"].ap())
        for ch in range(NPC // 512):
            sl = slice(ch * 512, (ch + 1) * 512)
            ps = phaps.tile([128, 512], F32, tag="ps")
            nc.tensor.matmul(ps, w16["wl1a"], iown[:, sl], start=True, stop=False)
            nc.tensor.matmul(ps, w16["wl1b"], sown[:, sl], start=False, stop=True)
            nc.scalar.activation(ia_own[:, sl], ps, ACT.Identity, bias=bias["bl1"])
            ps2 = phaps.tile([128, 512], F32, tag="ps")
            qT = phas.tile([128, 512], F16, tag="qT")
            nc.tensor.matmul(ps2, w16["wl3"], ia_own[:, sl])
            nc.scalar.activation(qT, ps2, ACT.Identity, bias=bias["bl3"])
            ps3 = phaps.tile([128, 512], F32, tag="ps")
            qt1 = phas.tile([128, 512], F16, tag="qt1")
            nc.tensor.matmul(ps3, w16["wq1"], qT)
            nc.scalar.activation(qt1, ps3, ACT.Copy)
            ps5 = phaps.tile([128, 512], F32, tag="ps")
            nc.tensor.matmul(ps5, w16["wWuraw"], qt1)   # (Qt1@Wu)^T
            nc.vector.tensor_copy(qt1t[:, sl], ps5)
            ps4 = phaps.tile([128, 512], F32, tag="ps")
            nc.tensor.matmul(ps4, w16["wq3"], qT)
            nc.vector.tensor_copy(qt3t[:, sl], ps4)
        for tt in range(NT):
            pst = phatps.tile([128, 128], F16, tag="qtt")
            nc.tensor.transpose(pst, qt1t[:, tt * 128:(tt + 1) * 128], ident)
            nc.vector.tensor_copy(qt1n[:, tt, :], pst)
            pst2 = phatps.tile([128, 128], F16, tag="qtt")
            nc.tensor.transpose(pst2, qt3t[:, tt * 128:(tt + 1) * 128], ident)
            nc.vector.tensor_copy(qt3n[:, tt, :], pst2)

    # ---------------- Phases C + B interleaved (shared pool) ----------------
    with tc.tile_pool(name="work", bufs=1) as work, \
         tc.tile_pool(name="phC_ps", bufs=2, space="PSUM") as phcps:

        def emit_c_chunk(chi):
            sl = slice(chi * UCH, (chi + 1) * UCH)
            rsb = work.tile([128, L, UCH], F16, tag="rsb", name="rsb", bufs=1)
            nc.sync.dma_start(out=rsb,
                              in_=t["rT"].ap()[:, :, sl].rearrange("t p u -> p t u"))
            k = work.tile([128, UCH], F16, tag="k", name="k", bufs=4)
            nc.vector.memset(k, 0.0)
            nc.vector.tensor_scalar_add(k, k, bias["kini"])
            HL = L // 2
            for st in range(L):
                if st % HL == 0:
                    h = st // HL
                    hs = slice(h * HL, (h + 1) * HL)
                    ifg = work.tile([128, HL, UCH], F16, tag="ifg", name="ifg",
                                    bufs=1)
                    nc.sync.dma_start(out=ifg, in_=t["ifgT"].ap()[chi][:, hs, :])
                    istg = work.tile([128, HL, UCH], F16, tag="istg", name="istg",
                                     bufs=1)
                    nc.sync.dma_start(out=istg, in_=t["istgT"].ap()[chi][:, hs, :])
                    sklg = work.tile([128, HL, UCH], F16, tag="sklg", name="sklg",
                                     bufs=1)
                    nc.sync.dma_start(out=sklg, in_=t["sklgT"].ap()[chi][:, hs, :])
                sh = st % HL
                psA = phcps.tile([128, UCH], F32, tag="psA")
                nc.tensor.matmul(psA, w16["wAW"], ifg[:, sh, :],
                                 start=True, stop=False)
                nc.tensor.matmul(psA, w16["wW1"], istg[:, sh, :],
                                 start=False, stop=False)
                nc.tensor.matmul(psA, w16["wW2"], sklg[:, sh, :],
                                 start=False, stop=False)
                nc.tensor.matmul(psA, w16["wC"], k, start=False, stop=True)
                q1 = work.tile([128, UCH], F16, tag="q1", name="q1", bufs=3)
                nc.scalar.activation(q1, psA, ACT.Identity, bias=bias["bq1"])
                psB = phcps.tile([128, UCH], F32, tag="psB")
                nc.tensor.matmul(psB, w16["wLo2"], rsb[:, st, :],
                                 start=True, stop=False)
                nc.tensor.matmul(psB, w16["wLo1"], q1, start=False, stop=True)
                psC = phcps.tile([128, UCH], F32, tag="psC")
                nc.tensor.matmul(psC, w16["wLq"], q1)
                psD = phcps.tile([128, UCH], F32, tag="psD")
                nc.tensor.matmul(psD, w16["wfo1"], rsb[:, st, :],
                                 start=True, stop=False)
                nc.tensor.matmul(psD, w16["wfo2"], k, start=False, stop=True)
                sg = work.tile([128, UCH], F16, tag="sg", name="sg", bufs=3)
                nc.scalar.activation(sg, psB, ACT.Sigmoid, bias=bias["bLo"])
                th = work.tile([128, UCH], F16, tag="th", name="th", bufs=3)
                nc.scalar.activation(th, psC, ACT.Tanh, bias=bias["bLq"])
                ff = work.tile([128, UCH], F16, tag="ff", name="ff", bufs=3)
                nc.scalar.activation(ff, psD, ACT.Sigmoid, bias=bias["bfo"])
                xx = work.tile([128, UCH], F16, tag="xx", name="xx", bufs=3)
                nc.vector.tensor_mul(xx, sg, th)
                d1 = work.tile([128, UCH], F16, tag="d1", name="d1", bufs=3)
                nc.vector.tensor_sub(d1, k, xx)
                d2 = work.tile([128, UCH], F16, tag="d2", name="d2", bufs=3)
                nc.vector.tensor_mul(d2, ff, d1)
                nc.vector.tensor_add(k, d2, xx)
            nc.sync.dma_start(out=t["out_u"].ap()[:, sl], in_=k)

        def emit_b_tile(tt):
            ug = work.tile([128, L, D], F16, tag="ug", name="ug", bufs=2)
            nc.sync.dma_start(out=ug, in_=t["ufg"].ap()[tt])
            rsp = work.tile([128, L, D], F16, tag="rsp", name="rsp", bufs=2)
            nc.sync.dma_start(out=rsp, in_=t["iresp"].ap()[tt])
            p1 = work.tile([128, L, D], F16, tag="p1", name="p1", bufs=3)
            nc.vector.tensor_mul(p1, ug, _bcast(qt1n[:, tt, :], L, 0))
            p2 = work.tile([128, L, D], F16, tag="p1", name="p2", bufs=3)
            nc.vector.tensor_mul(p2, rsp, _bcast(qt3n[:, tt, :], L, 0))
            s1 = work.tile([128, L, D], F16, tag="p1", name="s1", bufs=3)
            nc.vector.tensor_add(s1, p1, p2)
            s2 = work.tile([128, L, D // 2], F16, tag="s2", name="s2", bufs=2)
            nc.vector.tensor_add(s2, s1[:, :, 0:D // 2], s1[:, :, D // 2:D])
            s3 = work.tile([128, L, D // 4], F16, tag="s3", name="s3", bufs=2)
            nc.vector.tensor_add(s3, s2[:, :, 0:D // 4], s2[:, :, D // 4:D // 2])
            e = work.tile([128, L], F16, tag="e", name="e", bufs=2)
            nc.vector.tensor_reduce(e, s3[:], axis=mybir.AxisListType.X,
                                    op=mybir.AluOpType.add)
            m = work.tile([128, 1], F32, tag="m", name="m", bufs=2)
            nc.vector.tensor_reduce(m, e[:], axis=mybir.AxisListType.X,
                                    op=mybir.AluOpType.max)
            nm = work.tile([128, 1], F32, tag="nm", name="nm", bufs=2)
            nc.vector.tensor_scalar_mul(nm, m, -1.0 / SQD)
            al = work.tile([128, L], F32, tag="al", name="al", bufs=2)
            ssum = work.tile([128, 1], F32, tag="ssum", name="ssum", bufs=2)
            nc.scalar.activation(al, e, ACT.Exp, bias=nm, scale=1.0 / SQD,
                                 accum_out=ssum)
            rinv = work.tile([128, 1], F32, tag="rinv", name="rinv", bufs=2)
            nc.vector.reciprocal(rinv, ssum)
            al16 = work.tile([128, L], F16, tag="al16", name="al16", bufs=2)
            nc.vector.tensor_scalar_mul(al16, al, rinv)
            p3 = work.tile([128, L, D], F16, tag="p1", name="p3", bufs=3)
            nc.vector.tensor_mul(p3, rsp, _bcast(al16[:], D, 1))
            p3v = bass.AP(tensor=p3.tensor, offset=p3.offset,
                          ap=[list(p3.ap[0]), [1, D], [D, L]])
            nc.vector.tensor_reduce(rball[:, tt, :], p3v,
                                    axis=mybir.AxisListType.X,
                                    op=mybir.AluOpType.add, opt_input=False)

        TPC = NT // NCH
        for chi in range(NCH):
            emit_c_chunk(chi)
            for tt in range(TPC * chi, TPC * chi + TPC):
                emit_b_tile(tt)

    # ---------------- Phase B tail: rbar transpose + item output ----------------
    with tc.tile_pool(name="phBt_s", bufs=2) as phbts, \
         tc.tile_pool(name="phBt_ps", bufs=2, space="PSUM") as phbtps:
        for tt in range(NT):
            pst = phbtps.tile([128, 128], F16, tag="rbT")
            nc.tensor.transpose(pst, rball[:, tt, :], ident)
            rbT = phbts.tile([128, 128], F16, tag="rbTs")
            nc.vector.tensor_copy(rbT, pst)
            psI = phbtps.tile([128, 128], F32, tag="psI")
            nc.tensor.matmul(psI, w16["wl4a"], ia_own[:, tt * 128:(tt + 1) * 128],
                             start=True, stop=False)
            nc.tensor.matmul(psI, w16["wl4b"], rbT, start=False, stop=True)
            nc.scalar.activation(oi_sb[:, tt * 128:(tt + 1) * 128], psI,
                                 ACT.Identity, bias=bias["bl4"])
        nc.sync.dma_start(out=t["out_i"].ap(), in_=oi_sb)


def build_program():
    if "nc" in _CACHE:
        return _CACHE["nc"], _CACHE["names"]
    nc = bacc.Bacc("TRN2", target_bir_lowering=False, debug=False,
                   num_devices=NCORES)
    t = {}

    def din(name, shape, dtype):
        t[name] = nc.dram_tensor(name, shape, dtype, kind="ExternalInput")

    for n_ in ("wl1a", "wl1b", "wl3", "wq1", "wq3", "wl4a", "wl4b",
               "wLo2", "wfo1", "wC", "wfo2", "wLo1", "wLq",
               "wWuraw", "wAW", "wW1", "wW2"):
        din(n_, [D, D], F16)
    for n_ in ("bq1", "bLo", "bLq", "bfo", "bl1", "bl3", "bl4", "kini"):
        din(n_, [D, 1], F32)
    din("ist_own", [128, NPC], F16)
    din("skl_own", [128, NPC], F16)
    din("ufg", [NT, 128, L * D], F16)     # user_feat[item mailbox idx], natural
    din("iresp", [NT, 128, L * D], F16)
    din("rT", [L, 128, NPC], F16)         # user_mail_resp, transposed
    din("ifgT", [NCH, 128, L, UCH], F16)  # item_feat[user mailbox idx], transposed
    din("istgT", [NCH, 128, L, UCH], F16)
    din("sklgT", [NCH, 128, L, UCH], F16)
    t["out_u"] = nc.dram_tensor("out_u", [128, NPC], F16, kind="ExternalOutput")
    t["out_i"] = nc.dram_tensor("out_i", [128, NPC], F32, kind="ExternalOutput")

    with tile.TileContext(nc) as tc:
        with ExitStack() as ctx:
            _emit(ctx, tc, t)
    nc.compile()
    _CACHE["nc"] = nc
    _CACHE["names"] = t
    return nc, t


def prepare_inputs(user_feat, item_feat, user_static, item_static, skill,
                   user_mail_item_idx, user_mail_resp, item_mail_user_idx,
                   item_mail_resp, params):
    p = {k: np.asarray(v, dtype=np.float32) for k, v in params.items()}
    f16 = np.float16

    A, B = p["q1_W"][:, :D], p["q1_W"][:, D:2 * D]
    AW = A @ p["Wi"]                      # folded (A @ Wi)
    W1 = B @ p["l1_W"][:, :D]
    W2 = B @ p["l1_W"][:, D:]
    bq1f = p["q1_b"] + B @ p["l1_b"]      # folded bias

    rep = {
        "wl1a": np.ascontiguousarray(p["l1_W"][:, :D].T, dtype=f16),
        "wl1b": np.ascontiguousarray(p["l1_W"][:, D:].T, dtype=f16),
        "wl3": np.ascontiguousarray(p["l3_W"].T, dtype=f16),
        "wq1": np.ascontiguousarray(p["l2_W"][:, :D], dtype=f16),
        "wq3": np.ascontiguousarray(p["l2_W"][:, 2 * D:], dtype=f16),
        "wl4a": np.ascontiguousarray(p["l4_W"][:, :D].T, dtype=f16),
        "wl4b": np.ascontiguousarray(p["l4_W"][:, D:].T, dtype=f16),
        "wLo2": np.ascontiguousarray(p["Lo_W"][:, D:].T, dtype=f16),
        "wfo1": np.ascontiguousarray(p["fo_W"][:, :D].T, dtype=f16),
        "wC": np.ascontiguousarray(p["q1_W"][:, 2 * D:].T, dtype=f16),
        "wfo2": np.ascontiguousarray(p["fo_W"][:, D:].T, dtype=f16),
        "wLo1": np.ascontiguousarray(p["Lo_W"][:, :D].T, dtype=f16),
        "wLq": np.ascontiguousarray(p["Lq_W"].T, dtype=f16),
        "wWuraw": np.ascontiguousarray(p["Wu"], dtype=f16),
        "wAW": np.ascontiguousarray(AW.T, dtype=f16),
        "wW1": np.ascontiguousarray(W1.T, dtype=f16),
        "wW2": np.ascontiguousarray(W2.T, dtype=f16),
        "bq1": np.ascontiguousarray(bq1f[:, None], dtype=np.float32),
        "bLo": np.ascontiguousarray(p["Lo_b"][:, None], dtype=np.float32),
        "bLq": np.ascontiguousarray(p["Lq_b"][:, None], dtype=np.float32),
        "bfo": np.ascontiguousarray(p["fo_b"][:, None], dtype=np.float32),
        "bl1": np.ascontiguousarray(p["l1_b"][:, None], dtype=np.float32),
        "bl3": np.ascontiguousarray(p["l3_b"][:, None], dtype=np.float32),
        "bl4": np.ascontiguousarray(p["l4_b"][:, None], dtype=np.float32),
        "kini": np.ascontiguousarray(p["k_init"][0][:, None], dtype=np.float32),
    }

    uidx_full = np.asarray(user_mail_item_idx).astype(np.int64)
    iidx_full = np.asarray(item_mail_user_idx).astype(np.int64)
    uresp = np.asarray(user_mail_resp, dtype=np.float32)
    iresp = np.asarray(item_mail_resp, dtype=np.float32)
    uf16 = np.asarray(user_feat).astype(f16)
    if16 = np.asarray(item_feat).astype(f16)
    ist16 = np.asarray(item_static).astype(f16)
    skl16 = np.asarray(skill).astype(f16)
    ist_T = np.asarray(item_static).T
    skl_T = np.asarray(skill).T

    def stage_user_gather(tab16, idx):
        # [1024, L, D] -> [NCH, 128(d), L, UCH] (transposed, per chunk)
        g = tab16[idx]                                  # [1024, L, D]
        g = g.reshape(NCH, UCH, L, D).transpose(0, 3, 2, 1)
        return np.ascontiguousarray(g)

    in_maps = []
    for c in range(NCORES):
        s = slice(c * NPC, (c + 1) * NPC)
        ui = uidx_full[s]
        m = {
            "ist_own": np.ascontiguousarray(ist_T[:, s], dtype=f16),
            "skl_own": np.ascontiguousarray(skl_T[:, s], dtype=f16),
            "ufg": np.ascontiguousarray(
                uf16[iidx_full[s]].reshape(NT, 128, L * D)),
            "iresp": np.ascontiguousarray(
                iresp[s].reshape(NT, 128, L * D), dtype=f16),
            "rT": np.ascontiguousarray(uresp[s].transpose(1, 2, 0), dtype=f16),
            "ifgT": stage_user_gather(if16, ui),
            "istgT": stage_user_gather(ist16, ui),
            "sklgT": stage_user_gather(skl16, ui),
        }
        m.update(rep)
        in_maps.append(m)
    return in_maps


def kernel(user_feat, item_feat, user_static, item_static, skill,
           user_mail_item_idx, user_mail_resp, item_mail_user_idx,
           item_mail_resp, params):
    nc, _ = build_program()
    in_maps = prepare_inputs(
        user_feat, item_feat, user_static, item_static, skill,
        user_mail_item_idx, user_mail_resp, item_mail_user_idx,
        item_mail_resp, params)
    trace = os.environ.get("KERNEL_TRACE", "0") == "1"
    res = bass_utils.run_bass_kernel_spmd(
        nc, in_maps, core_ids=list(range(NCORES)), trace=trace)
    _CACHE["last_result"] = res
    user_new = np.concatenate(
        [res.results[c]["out_u"].T for c in range(NCORES)], axis=0)
    item_new = np.concatenate(
        [res.results[c]["out_i"].T for c in range(NCORES)], axis=0)
    return (np.ascontiguousarray(user_new, dtype=np.float32),
            np.ascontiguousarray(item_new, dtype=np.float32))


# revision 26
# speedup vs baseline: 1.3497x; 1.0059x over previous
"""Trainium2 Bass kernel for nn_BDGKTLayers (GNN message passing).

Host wrapper: shards the 8192-node user/item graphs across 8 NeuronCores
(1024 users + 1024 items per core), stages layout-transformed fp16 inputs,
runs one SPMD Bass/Tile program per core, reassembles full outputs.

Gathers are hoisted to the host as pure row-copies of RAW inputs (no host
FLOPs) by commuting them with the linear projections, which are folded
into weights:
  item attention logits  e[i,l] = Qt1[i]*ud[uidx[i,l]] + Qt3[i]*resp[i,l]
       with ud = uf@Wu.T  ->  (Qt1@Wu)[i] * uf[uidx[i,l]]
  user recurrence per-edge input T1[idx] = (A@Wi)@if[idx] + (B@l1a)@ist[idx]
       + (B@l1b)@skl[idx] (+ folded bias), staged transposed.
The big per-edge "key"/"V" matmuls of the reference are eliminated
algebraically (softmax shift invariance; V-sum linearity).  All model
FLOPs (projections, attention reduce, softmax, 32-step gated recurrence)
run on device; fp16 operands, fp32 psum accumulation, fp16 state
(validated ~8e-4 rel err vs fp32 reference).
"""

import os
import sys

sys.path.insert(0, "/opt/trn_rl_repo")

from contextlib import ExitStack

import numpy as np

import concourse.bass as bass
import concourse.tile as tile
from concourse import bacc, mybir
from concourse import bass_utils
from concourse.masks import make_identity

F16 = mybir.dt.float16
F32 = mybir.dt.float32
I16 = mybir.dt.int16

NN = 8192          # nodes (users == items)
D = 128
L = 32             # mailbox length
NCORES = 8
NPC = NN // NCORES  # nodes per core = 1024
NT = NPC // 128     # 128-row item tiles per core = 8
UCH = 512           # user chunk (matmul free dim)
NCH = NPC // UCH    # chunks per core = 2
SQD = float(np.sqrt(D))

_CACHE = {}


def _bcast(ap2d, n, pos):
    """Insert a 0-stride dim of size n into a 2D AP at free position pos."""
    new = [list(ap2d.ap[0])]
    free = [list(p) for p in ap2d.ap[1:]]
    free.insert(pos, [0, n])
    return bass.AP(tensor=ap2d.tensor, offset=ap2d.offset, ap=new + free)


def _emit(ctx: ExitStack, tc: tile.TileContext, t):
    nc = tc.nc
    ctx.enter_context(nc.allow_low_precision("fp16 logit/rbar accumulation, validated"))
    ACT = mybir.ActivationFunctionType

    consts = ctx.enter_context(tc.tile_pool(name="consts", bufs=1))

    def load_const(name, shape, dtype):
        s = consts.tile(shape, dtype, tag=name, name=name)
        nc.sync.dma_start(out=s, in_=t[name].ap())
        return s

    w16 = {}
    for n_ in ("wl1a", "wl1b", "wl3", "wq1", "wq3", "wl4a", "wl4b",
               "wLo2", "wfo1", "wC", "wfo2", "wLo1", "wLq",
               "wWuraw", "wAW", "wW1", "wW2"):
        w16[n_] = load_const(n_, [D, D], F16)
    bias = {}
    for n_ in ("bq1", "bLo", "bLq", "bfo", "bl1", "bl3", "bl4", "kini"):
        bias[n_] = load_const(n_, [D, 1], F32)
    ident = consts.tile([D, D], F16, tag="ident")
    make_identity(nc, ident)
    ident32 = consts.tile([D, D], F32, tag="ident32")
    make_identity(nc, ident32)

    keep = ctx.enter_context(tc.tile_pool(name="keep", bufs=1))
    ia_own = keep.tile([128, NPC], F16, tag="ia_own")
    qt1n = keep.tile([128, NT, D], F16, tag="qt1n")   # (Qt1@Wu), natural
    qt3n = keep.tile([128, NT, D], F16, tag="qt3n")
    oi_sb = keep.tile([128, NPC], F32, tag="oi_sb")
    rball = keep.tile([128, NT, D], F16, tag="rball")

    # ---------------- Phase A: own-item tables ----------------
    with tc.tile_pool(name="phA", bufs=1) as pha, \
         tc.tile_pool(name="phA_ps", bufs=2, space="PSUM") as phaps, \
         tc.tile_pool(name="phA_tps", bufs=2, space="PSUM") as phatps, \
         tc.tile_pool(name="phA_s", bufs=3) as phas:
        qt1t = pha.tile([128, NPC], F16, tag="qt1t")
        qt3t = pha.tile([128, NPC], F16, tag="qt3t")
        iown = pha.tile([128, NPC], F16, tag="iown")
        sown = pha.tile([128, NPC], F16, tag="sown")
        nc.sync.dma_start(out=iown, in_=t["ist_own"].ap())
        nc.sync.dma_start(out=sown, in_=t["skl_own"].ap())
        for ch in range(NPC // 512):
            sl = slice(ch * 512, (ch + 1) * 512)
            ps = phaps.tile([128, 512], F32, tag="ps")
            nc.tensor.matmul(ps, w16["wl1a"], iown[:, sl], start=True, stop=False)
            nc.tensor.matmul(ps, w16["wl1b"], sown[:, sl], start=False, stop=True)
            nc.scalar.activation(ia_own[:, sl], ps, ACT.Identity, bias=bias["bl1"])
            ps2 = phaps.tile([128, 512], F32, tag="ps")
            qT = phas.tile([128, 512], F16, tag="qT")
            nc.tensor.matmul(ps2, w16["wl3"], ia_own[:, sl])
            nc.scalar.activation(qT, ps2, ACT.Identity, bias=bias["bl3"])
            ps3 = phaps.tile([128, 512], F32, tag="ps")
            qt1 = phas.tile([128, 512], F16, tag="qt1")
            nc.tensor.matmul(ps3, w16["wq1"], qT)
            nc.scalar.activation(qt1, ps3, ACT.Copy)
            ps5 = phaps.tile([128, 512], F32, tag="ps")
            nc.tensor.matmul(ps5, w16["wWuraw"], qt1)   # (Qt1@Wu)^T
            nc.vector.tensor_copy(qt1t[:, sl], ps5)
            ps4 = phaps.tile([128, 512], F32, tag="ps")
            nc.tensor.matmul(ps4, w16["wq3"], qT)
            nc.vector.tensor_copy(qt3t[:, sl], ps4)
        for tt in range(NT):
            pst = phatps.tile([128, 128], F16, tag="qtt")
            nc.tensor.transpose(pst, qt1t[:, tt * 128:(tt + 1) * 128], ident)
            nc.vector.tensor_copy(qt1n[:, tt, :], pst)
            pst2 = phatps.tile([128, 128], F16, tag="qtt")
            nc.tensor.transpose(pst2, qt3t[:, tt * 128:(tt + 1) * 128], ident)
            nc.vector.tensor_copy(qt3n[:, tt, :], pst2)

    # ---------------- Phases C + B interleaved (shared pool) ----------------
    with tc.tile_pool(name="work", bufs=1) as work, \
         tc.tile_pool(name="phC_ps", bufs=2, space="PSUM") as phcps:

        def emit_c_chunk(chi):
            sl = slice(chi * UCH, (chi + 1) * UCH)
            k = work.tile([128, UCH], F16, tag="k", name="k", bufs=4)
            nc.vector.memset(k, 0.0)
            nc.vector.tensor_scalar_add(k, k, bias["kini"])
            HL = L // 2
            for st in range(L):
                if st % HL == 0:
                    h = st // HL
                    hs = slice(h * HL, (h + 1) * HL)
                    ifg = work.tile([128, HL, UCH], F16, tag="ifg", name="ifg",
                                    bufs=1)
                    nc.sync.dma_start(out=ifg, in_=t["ifgT"].ap()[chi][:, hs, :])
                    istg = work.tile([128, HL, UCH], F16, tag="istg", name="istg",
                                     bufs=1)
                    nc.sync.dma_start(out=istg, in_=t["istgT"].ap()[chi][:, hs, :])
                    sklg = work.tile([128, HL, UCH], F16, tag="sklg", name="sklg",
                                     bufs=1)
                    nc.sync.dma_start(out=sklg, in_=t["sklgT"].ap()[chi][:, hs, :])
                    rsb = work.tile([128, HL, UCH], F16, tag="rsb", name="rsb",
                                    bufs=2)
                    nc.sync.dma_start(
                        out=rsb,
                        in_=t["rT"].ap()[hs, :, sl].rearrange("t p u -> p t u"))
                sh = st % HL
                psA = phcps.tile([128, UCH], F32, tag="psA")
                nc.tensor.matmul(psA, w16["wAW"], ifg[:, sh, :],
                                 start=True, stop=False)
                nc.tensor.matmul(psA, w16["wW1"], istg[:, sh, :],
                                 start=False, stop=False)
                nc.tensor.matmul(psA, w16["wW2"], sklg[:, sh, :],
                                 start=False, stop=False)
                nc.tensor.matmul(psA, w16["wC"], k, start=False, stop=True)
                q1 = work.tile([128, UCH], F16, tag="q1", name="q1", bufs=3)
                nc.scalar.activation(q1, psA, ACT.Identity, bias=bias["bq1"])
                psB = phcps.tile([128, UCH], F32, tag="psB")
                nc.tensor.matmul(psB, w16["wLo2"], rsb[:, sh, :],
                                 start=True, stop=False)
                nc.tensor.matmul(psB, w16["wLo1"], q1, start=False, stop=True)
                psC = phcps.tile([128, UCH], F32, tag="psC")
                nc.tensor.matmul(psC, w16["wLq"], q1)
                psD = phcps.tile([128, UCH], F32, tag="psD")
                nc.tensor.matmul(psD, w16["wfo1"], rsb[:, sh, :],
                                 start=True, stop=False)
                nc.tensor.matmul(psD, w16["wfo2"], k, start=False, stop=True)
                sg = work.tile([128, UCH], F16, tag="sg", name="sg", bufs=3)
                nc.scalar.activation(sg, psB, ACT.Sigmoid, bias=bias["bLo"])
                th = work.tile([128, UCH], F16, tag="th", name="th", bufs=3)
                nc.scalar.activation(th, psC, ACT.Tanh, bias=bias["bLq"])
                ff = work.tile([128, UCH], F16, tag="ff", name="ff", bufs=3)
                nc.scalar.activation(ff, psD, ACT.Sigmoid, bias=bias["bfo"])
                xx = work.tile([128, UCH], F16, tag="xx", name="xx", bufs=3)
                nc.vector.tensor_mul(xx, sg, th)
                d1 = work.tile([128, UCH], F16, tag="d1", name="d1", bufs=3)
                nc.vector.tensor_sub(d1, k, xx)
                d2 = work.tile([128, UCH], F16, tag="d2", name="d2", bufs=3)
                nc.vector.tensor_mul(d2, ff, d1)
                nc.vector.tensor_add(k, d2, xx)
            nc.sync.dma_start(out=t["out_u"].ap()[:, sl], in_=k)

        def emit_b_tile(tt):
            ug = work.tile([128, L, D], F16, tag="ug", name="ug", bufs=2)
            nc.sync.dma_start(out=ug, in_=t["ufg"].ap()[tt])
            rsp = work.tile([128, L, D], F16, tag="rsp", name="rsp", bufs=2)
            nc.sync.dma_start(out=rsp, in_=t["iresp"].ap()[tt])
            p1 = work.tile([128, L, D], F16, tag="p1", name="p1", bufs=3)
            nc.vector.tensor_mul(p1, ug, _bcast(qt1n[:, tt, :], L, 0))
            p2 = work.tile([128, L, D], F16, tag="p1", name="p2", bufs=3)
            nc.vector.tensor_mul(p2, rsp, _bcast(qt3n[:, tt, :], L, 0))
            s1 = work.tile([128, L, D], F16, tag="p1", name="s1", bufs=3)
            nc.vector.tensor_add(s1, p1, p2)
            s2 = work.tile([128, L, D // 2], F16, tag="s2", name="s2", bufs=2)
            nc.vector.tensor_add(s2, s1[:, :, 0:D // 2], s1[:, :, D // 2:D])
            s3 = work.tile([128, L, D // 4], F16, tag="s3", name="s3", bufs=2)
            nc.vector.tensor_add(s3, s2[:, :, 0:D // 4], s2[:, :, D // 4:D // 2])
            e = work.tile([128, L], F16, tag="e", name="e", bufs=2)
            nc.vector.tensor_reduce(e, s3[:], axis=mybir.AxisListType.X,
                                    op=mybir.AluOpType.add)
            m = work.tile([128, 1], F32, tag="m", name="m", bufs=2)
            nc.vector.tensor_reduce(m, e[:], axis=mybir.AxisListType.X,
                                    op=mybir.AluOpType.max)
            nm = work.tile([128, 1], F32, tag="nm", name="nm", bufs=2)
            nc.vector.tensor_scalar_mul(nm, m, -1.0 / SQD)
            al = work.tile([128, L], F32, tag="al", name="al", bufs=2)
            ssum = work.tile([128, 1], F32, tag="ssum", name="ssum", bufs=2)
            nc.scalar.activation(al, e, ACT.Exp, bias=nm, scale=1.0 / SQD,
                                 accum_out=ssum)
            rinv = work.tile([128, 1], F32, tag="rinv", name="rinv", bufs=2)
            nc.vector.reciprocal(rinv, ssum)
            al16 = work.tile([128, L], F16, tag="al16", name="al16", bufs=2)
            nc.vector.tensor_scalar_mul(al16, al, rinv)
            p3 = work.tile([128, L, D], F16, tag="p1", name="p3", bufs=3)
            nc.vector.tensor_mul(p3, rsp, _bcast(al16[:], D, 1))
            p3v = bass.AP(tensor=p3.tensor, offset=p3.offset,
                          ap=[list(p3.ap[0]), [1, D], [D, L]])
            nc.vector.tensor_reduce(rball[:, tt, :], p3v,
                                    axis=mybir.AxisListType.X,
                                    op=mybir.AluOpType.add, opt_input=False)

        TPC = NT // NCH
        for chi in range(NCH):
            emit_c_chunk(chi)
            for tt in range(TPC * chi, TPC * chi + TPC):
                emit_b_tile(tt)

    # ---------------- Phase B tail: rbar transpose + item output ----------------
    with tc.tile_pool(name="phBt_s", bufs=2) as phbts, \
         tc.tile_pool(name="phBt_ps", bufs=2, space="PSUM") as phbtps:
        for tt in range(NT):
            pst = phbtps.tile([128, 128], F16, tag="rbT")
            nc.tensor.transpose(pst, rball[:, tt, :], ident)
            rbT = phbts.tile([128, 128], F16, tag="rbTs")
            nc.vector.tensor_copy(rbT, pst)
            psI = phbtps.tile([128, 128], F32, tag="psI")
            nc.tensor.matmul(psI, w16["wl4a"], ia_own[:, tt * 128:(tt + 1) * 128],
                             start=True, stop=False)
            nc.tensor.matmul(psI, w16["wl4b"], rbT, start=False, stop=True)
            nc.scalar.activation(oi_sb[:, tt * 128:(tt + 1) * 128], psI,
                                 ACT.Identity, bias=bias["bl4"])
        nc.sync.dma_start(out=t["out_i"].ap(), in_=oi_sb)


def build_program():
    if "nc" in _CACHE:
        return _CACHE["nc"], _CACHE["names"]
    nc = bacc.Bacc("TRN2", target_bir_lowering=False, debug=False,
                   num_devices=NCORES)
    t = {}

    def din(name, shape, dtype):
        t[name] = nc.dram_tensor(name, shape, dtype, kind="ExternalInput")

    for n_ in ("wl1a", "wl1b", "wl3", "wq1", "wq3", "wl4a", "wl4b",
               "wLo2", "wfo1", "wC", "wfo2", "wLo1", "wLq",
               "wWuraw", "wAW", "wW1", "wW2"):
        din(n_, [D, D], F16)
    for n_ in ("bq1", "bLo", "bLq", "bfo", "bl1", "bl3", "bl4", "kini"):
        din(n_, [D, 1], F32)
    din("ist_own", [128, NPC], F16)
    din("skl_own", [128, NPC], F16)
    din("ufg", [NT, 128, L * D], F16)     # user_feat[item mailbox idx], natural
    din("iresp", [NT, 128, L * D], F16)
    din("rT", [L, 128, NPC], F16)         # user_mail_resp, transposed
    din("ifgT", [NCH, 128, L, UCH], F16)  # item_feat[user mailbox idx], transposed
    din("istgT", [NCH, 128, L, UCH], F16)
    din("sklgT", [NCH, 128, L, UCH], F16)
    t["out_u"] = nc.dram_tensor("out_u", [128, NPC], F16, kind="ExternalOutput")
    t["out_i"] = nc.dram_tensor("out_i", [128, NPC], F32, kind="ExternalOutput")

    with tile.TileContext(nc) as tc:
        with ExitStack() as ctx:
            _emit(ctx, tc, t)
    nc.compile()
    _CACHE["nc"] = nc
    _CACHE["names"] = t
    return nc, t


def prepare_inputs(user_feat, item_feat, user_static, item_static, skill,
                   user_mail_item_idx, user_mail_resp, item_mail_user_idx,
                   item_mail_resp, params):
    p = {k: np.asarray(v, dtype=np.float32) for k, v in params.items()}
    f16 = np.float16

    A, B = p["q1_W"][:, :D], p["q1_W"][:, D:2 * D]
    AW = A @ p["Wi"]                      # folded (A @ Wi)
    W1 = B @ p["l1_W"][:, :D]
    W2 = B @ p["l1_W"][:, D:]
    bq1f = p["q1_b"] + B @ p["l1_b"]      # folded bias

    rep = {
        "wl1a": np.ascontiguousarray(p["l1_W"][:, :D].T, dtype=f16),
        "wl1b": np.ascontiguousarray(p["l1_W"][:, D:].T, dtype=f16),
        "wl3": np.ascontiguousarray(p["l3_W"].T, dtype=f16),
        "wq1": np.ascontiguousarray(p["l2_W"][:, :D], dtype=f16),
        "wq3": np.ascontiguousarray(p["l2_W"][:, 2 * D:], dtype=f16),
        "wl4a": np.ascontiguousarray(p["l4_W"][:, :D].T, dtype=f16),
        "wl4b": np.ascontiguousarray(p["l4_W"][:, D:].T, dtype=f16),
        "wLo2": np.ascontiguousarray(p["Lo_W"][:, D:].T, dtype=f16),
        "wfo1": np.ascontiguousarray(p["fo_W"][:, :D].T, dtype=f16),
        "wC": np.ascontiguousarray(p["q1_W"][:, 2 * D:].T, dtype=f16),
        "wfo2": np.ascontiguousarray(p["fo_W"][:, D:].T, dtype=f16),
        "wLo1": np.ascontiguousarray(p["Lo_W"][:, :D].T, dtype=f16),
        "wLq": np.ascontiguousarray(p["Lq_W"].T, dtype=f16),
        "wWuraw": np.ascontiguousarray(p["Wu"], dtype=f16),
        "wAW": np.ascontiguousarray(AW.T, dtype=f16),
        "wW1": np.ascontiguousarray(W1.T, dtype=f16),
        "wW2": np.ascontiguousarray(W2.T, dtype=f16),
        "bq1": np.ascontiguousarray(bq1f[:, None], dtype=np.float32),
        "bLo": np.ascontiguousarray(p["Lo_b"][:, None], dtype=np.float32),
        "bLq": np.ascontiguousarray(p["Lq_b"][:, None], dtype=np.float32),
        "bfo": np.ascontiguousarray(p["fo_b"][:, None], dtype=np.float32),
        "bl1": np.ascontiguousarray(p["l1_b"][:, None], dtype=np.float32),
        "bl3": np.ascontiguousarray(p["l3_b"][:, None], dtype=np.float32),
        "bl4": np.ascontiguousarray(p["l4_b"][:, None], dtype=np.float32),
        "kini": np.ascontiguousarray(p["k_init"][0][:, None], dtype=np.float32),
    }

    uidx_full = np.asarray(user_mail_item_idx).astype(np.int64)
    iidx_full = np.asarray(item_mail_user_idx).astype(np.int64)
    uresp = np.asarray(user_mail_resp, dtype=np.float32)
    iresp = np.asarray(item_mail_resp, dtype=np.float32)
    uf16 = np.asarray(user_feat).astype(f16)
    if16 = np.asarray(item_feat).astype(f16)
    ist16 = np.asarray(item_static).astype(f16)
    skl16 = np.asarray(skill).astype(f16)
    ist_T = np.asarray(item_static).T
    skl_T = np.asarray(skill).T

    def stage_user_gather(tab16, idx):
        # [1024, L, D] -> [NCH, 128(d), L, UCH] (transposed, per chunk)
        g = tab16[idx]                                  # [1024, L, D]
        g = g.reshape(NCH, UCH, L, D).transpose(0, 3, 2, 1)
        return np.ascontiguousarray(g)

    in_maps = []
    for c in range(NCORES):
        s = slice(c * NPC, (c + 1) * NPC)
        ui = uidx_full[s]
        m = {
            "ist_own": np.ascontiguousarray(ist_T[:, s], dtype=f16),
            "skl_own": np.ascontiguousarray(skl_T[:, s], dtype=f16),
            "ufg": np.ascontiguousarray(
                uf16[iidx_full[s]].reshape(NT, 128, L * D)),
            "iresp": np.ascontiguousarray(
                iresp[s].reshape(NT, 128, L * D), dtype=f16),
            "rT": np.ascontiguousarray(uresp[s].transpose(1, 2, 0), dtype=f16),
            "ifgT": stage_user_gather(if16, ui),
            "istgT": stage_user_gather(ist16, ui),
            "sklgT": stage_user_gather(skl16, ui),
        }
        m.update(rep)
        in_maps.append(m)
    return in_maps


def kernel(user_feat, item_feat, user_static, item_static, skill,
           user_mail_item_idx, user_mail_resp, item_mail_user_idx,
           item_mail_resp, params):
    nc, _ = build_program()
    in_maps = prepare_inputs(
        user_feat, item_feat, user_static, item_static, skill,
        user_mail_item_idx, user_mail_resp, item_mail_user_idx,
        item_mail_resp, params)
    trace = os.environ.get("KERNEL_TRACE", "0") == "1"
    res = bass_utils.run_bass_kernel_spmd(
        nc, in_maps, core_ids=list(range(NCORES)), trace=trace)
    _CACHE["last_result"] = res
    user_new = np.concatenate(
        [res.results[c]["out_u"].T for c in range(NCORES)], axis=0)
    item_new = np.concatenate(
        [res.results[c]["out_i"].T for c in range(NCORES)], axis=0)
    return (np.ascontiguousarray(user_new, dtype=np.float32),
            np.ascontiguousarray(item_new, dtype=np.float32))


# revision 27
# speedup vs baseline: 1.3652x; 1.0115x over previous
"""Trainium2 Bass kernel for nn_BDGKTLayers (GNN message passing).

Host wrapper: shards the 8192-node user/item graphs across 8 NeuronCores
(1024 users + 1024 items per core), stages layout-transformed fp16 inputs,
runs one SPMD Bass/Tile program per core, reassembles full outputs.

Gathers are hoisted to the host as pure row-copies of RAW inputs (no host
FLOPs) by commuting them with the linear projections, which are folded
into weights:
  item attention logits  e[i,l] = Qt1[i]*ud[uidx[i,l]] + Qt3[i]*resp[i,l]
       with ud = uf@Wu.T  ->  (Qt1@Wu)[i] * uf[uidx[i,l]]
  user recurrence per-edge input T1[idx] = (A@Wi)@if[idx] + (B@l1a)@ist[idx]
       + (B@l1b)@skl[idx] (+ folded bias), staged transposed.
The big per-edge "key"/"V" matmuls of the reference are eliminated
algebraically (softmax shift invariance; V-sum linearity).  All model
FLOPs (projections, attention reduce, softmax, 32-step gated recurrence)
run on device; fp16 operands, fp32 psum accumulation, fp16 state
(validated ~8e-4 rel err vs fp32 reference).
"""

import os
import sys

sys.path.insert(0, "/opt/trn_rl_repo")

from contextlib import ExitStack

import numpy as np

import concourse.bass as bass
import concourse.tile as tile
from concourse import bacc, mybir
from concourse import bass_utils
from concourse.masks import make_identity

F16 = mybir.dt.float16
F32 = mybir.dt.float32
I16 = mybir.dt.int16

NN = 8192          # nodes (users == items)
D = 128
L = 32             # mailbox length
NCORES = 8
NPC = NN // NCORES  # nodes per core = 1024
NT = NPC // 128     # 128-row item tiles per core = 8
UCH = 512           # user chunk (matmul free dim)
NCH = NPC // UCH    # chunks per core = 2
SQD = float(np.sqrt(D))

_CACHE = {}


def _bcast(ap2d, n, pos):
    """Insert a 0-stride dim of size n into a 2D AP at free position pos."""
    new = [list(ap2d.ap[0])]
    free = [list(p) for p in ap2d.ap[1:]]
    free.insert(pos, [0, n])
    return bass.AP(tensor=ap2d.tensor, offset=ap2d.offset, ap=new + free)


def _emit(ctx: ExitStack, tc: tile.TileContext, t):
    nc = tc.nc
    ctx.enter_context(nc.allow_low_precision("fp16 logit/rbar accumulation, validated"))
    ACT = mybir.ActivationFunctionType

    consts = ctx.enter_context(tc.tile_pool(name="consts", bufs=1))

    def load_const(name, shape, dtype):
        s = consts.tile(shape, dtype, tag=name, name=name)
        nc.sync.dma_start(out=s, in_=t[name].ap())
        return s

    w16 = {}
    for n_ in ("wl1a", "wl1b", "wl3", "wq1", "wq3", "wl4a", "wl4b",
               "wLo2", "wfo1", "wC", "wfo2", "wLo1", "wLq",
               "wWuraw", "wAW", "wW1", "wW2"):
        w16[n_] = load_const(n_, [D, D], F16)
    bias = {}
    for n_ in ("bq1", "bLo", "bLq", "bfo", "bl1", "bl3", "bl4", "kini"):
        bias[n_] = load_const(n_, [D, 1], F32)
    ident = consts.tile([D, D], F16, tag="ident")
    make_identity(nc, ident)
    ident32 = consts.tile([D, D], F32, tag="ident32")
    make_identity(nc, ident32)

    keep = ctx.enter_context(tc.tile_pool(name="keep", bufs=1))
    ia_own = keep.tile([128, NPC], F16, tag="ia_own")
    qt1n = keep.tile([128, NT, D], F16, tag="qt1n")   # (Qt1@Wu), natural
    qt3n = keep.tile([128, NT, D], F16, tag="qt3n")
    oi_sb = keep.tile([128, NPC], F32, tag="oi_sb")
    rball = keep.tile([128, NT, D], F16, tag="rball")

    # ---------------- Phase A: own-item tables ----------------
    with tc.tile_pool(name="phA", bufs=1) as pha, \
         tc.tile_pool(name="phA_ps", bufs=2, space="PSUM") as phaps, \
         tc.tile_pool(name="phA_tps", bufs=2, space="PSUM") as phatps, \
         tc.tile_pool(name="phA_s", bufs=3) as phas:
        qt1t = pha.tile([128, NPC], F16, tag="qt1t")
        qt3t = pha.tile([128, NPC], F16, tag="qt3t")
        iown = pha.tile([128, NPC], F16, tag="iown")
        sown = pha.tile([128, NPC], F16, tag="sown")
        nc.sync.dma_start(out=iown, in_=t["ist_own"].ap())
        nc.sync.dma_start(out=sown, in_=t["skl_own"].ap())
        for ch in range(NPC // 512):
            sl = slice(ch * 512, (ch + 1) * 512)
            ps = phaps.tile([128, 512], F32, tag="ps")
            nc.tensor.matmul(ps, w16["wl1a"], iown[:, sl], start=True, stop=False)
            nc.tensor.matmul(ps, w16["wl1b"], sown[:, sl], start=False, stop=True)
            nc.scalar.activation(ia_own[:, sl], ps, ACT.Identity, bias=bias["bl1"])
            ps2 = phaps.tile([128, 512], F32, tag="ps")
            qT = phas.tile([128, 512], F16, tag="qT")
            nc.tensor.matmul(ps2, w16["wl3"], ia_own[:, sl])
            nc.scalar.activation(qT, ps2, ACT.Identity, bias=bias["bl3"])
            ps3 = phaps.tile([128, 512], F32, tag="ps")
            qt1 = phas.tile([128, 512], F16, tag="qt1")
            nc.tensor.matmul(ps3, w16["wq1"], qT)
            nc.scalar.activation(qt1, ps3, ACT.Copy)
            ps5 = phaps.tile([128, 512], F32, tag="ps")
            nc.tensor.matmul(ps5, w16["wWuraw"], qt1)   # (Qt1@Wu)^T
            nc.vector.tensor_copy(qt1t[:, sl], ps5)
            ps4 = phaps.tile([128, 512], F32, tag="ps")
            nc.tensor.matmul(ps4, w16["wq3"], qT)
            nc.vector.tensor_copy(qt3t[:, sl], ps4)
        for tt in range(NT):
            pst = phatps.tile([128, 128], F16, tag="qtt")
            nc.tensor.transpose(pst, qt1t[:, tt * 128:(tt + 1) * 128], ident)
            nc.vector.tensor_copy(qt1n[:, tt, :], pst)
            pst2 = phatps.tile([128, 128], F16, tag="qtt")
            nc.tensor.transpose(pst2, qt3t[:, tt * 128:(tt + 1) * 128], ident)
            nc.vector.tensor_copy(qt3n[:, tt, :], pst2)

    # ---------------- Phases C + B interleaved (shared pool) ----------------
    with tc.tile_pool(name="work", bufs=1) as work, \
         tc.tile_pool(name="phC_ps", bufs=2, space="PSUM") as phcps:

        def emit_c_chunk(chi):
            sl = slice(chi * UCH, (chi + 1) * UCH)
            k = work.tile([128, UCH], F16, tag="k", name="k", bufs=4)
            nc.vector.memset(k, 0.0)
            nc.vector.tensor_scalar_add(k, k, bias["kini"])
            HL = L // 2
            for st in range(L):
                if st % HL == 0:
                    h = st // HL
                    hs = slice(h * HL, (h + 1) * HL)
                    ifg = work.tile([128, HL, UCH], F16, tag="ifg", name="ifg",
                                    bufs=1)
                    nc.sync.dma_start(out=ifg, in_=t["ifgT"].ap()[chi][:, hs, :])
                    istg = work.tile([128, HL, UCH], F16, tag="istg", name="istg",
                                     bufs=1)
                    nc.sync.dma_start(out=istg, in_=t["istgT"].ap()[chi][:, hs, :])
                    sklg = work.tile([128, HL, UCH], F16, tag="sklg", name="sklg",
                                     bufs=1)
                    nc.sync.dma_start(out=sklg, in_=t["sklgT"].ap()[chi][:, hs, :])
                    rsb = work.tile([128, HL, UCH], F16, tag="rsb", name="rsb",
                                    bufs=2)
                    nc.sync.dma_start(
                        out=rsb,
                        in_=t["rT"].ap()[hs, :, sl].rearrange("t p u -> p t u"))
                sh = st % HL
                psA = phcps.tile([128, UCH], F32, tag="psA")
                nc.tensor.matmul(psA, w16["wAW"], ifg[:, sh, :],
                                 start=True, stop=False)
                nc.tensor.matmul(psA, w16["wW1"], istg[:, sh, :],
                                 start=False, stop=False)
                nc.tensor.matmul(psA, w16["wW2"], sklg[:, sh, :],
                                 start=False, stop=False)
                nc.tensor.matmul(psA, w16["wC"], k, start=False, stop=True)
                q1 = work.tile([128, UCH], F16, tag="q1", name="q1", bufs=4)
                nc.scalar.activation(q1, psA, ACT.Identity, bias=bias["bq1"])
                psB = phcps.tile([128, UCH], F32, tag="psB")
                nc.tensor.matmul(psB, w16["wLo2"], rsb[:, sh, :],
                                 start=True, stop=False)
                nc.tensor.matmul(psB, w16["wLo1"], q1, start=False, stop=True)
                psC = phcps.tile([128, UCH], F32, tag="psC")
                nc.tensor.matmul(psC, w16["wLq"], q1)
                psD = phcps.tile([128, UCH], F32, tag="psD")
                nc.tensor.matmul(psD, w16["wfo1"], rsb[:, sh, :],
                                 start=True, stop=False)
                nc.tensor.matmul(psD, w16["wfo2"], k, start=False, stop=True)
                sg = work.tile([128, UCH], F16, tag="sg", name="sg", bufs=4)
                nc.scalar.activation(sg, psB, ACT.Sigmoid, bias=bias["bLo"])
                th = work.tile([128, UCH], F16, tag="th", name="th", bufs=4)
                nc.scalar.activation(th, psC, ACT.Tanh, bias=bias["bLq"])
                ff = work.tile([128, UCH], F16, tag="ff", name="ff", bufs=4)
                nc.scalar.activation(ff, psD, ACT.Sigmoid, bias=bias["bfo"])
                xx = work.tile([128, UCH], F16, tag="xx", name="xx", bufs=4)
                nc.vector.tensor_mul(xx, sg, th)
                d1 = work.tile([128, UCH], F16, tag="d1", name="d1", bufs=4)
                nc.vector.tensor_sub(d1, k, xx)
                d2 = work.tile([128, UCH], F16, tag="d2", name="d2", bufs=4)
                nc.vector.tensor_mul(d2, ff, d1)
                nc.vector.tensor_add(k, d2, xx)
            nc.sync.dma_start(out=t["out_u"].ap()[:, sl], in_=k)

        def emit_b_tile(tt):
            ug = work.tile([128, L, D], F16, tag="ug", name="ug", bufs=2)
            nc.sync.dma_start(out=ug, in_=t["ufg"].ap()[tt])
            rsp = work.tile([128, L, D], F16, tag="rsp", name="rsp", bufs=2)
            nc.sync.dma_start(out=rsp, in_=t["iresp"].ap()[tt])
            p1 = work.tile([128, L, D], F16, tag="p1", name="p1", bufs=3)
            nc.vector.tensor_mul(p1, ug, _bcast(qt1n[:, tt, :], L, 0))
            p2 = work.tile([128, L, D], F16, tag="p1", name="p2", bufs=3)
            nc.vector.tensor_mul(p2, rsp, _bcast(qt3n[:, tt, :], L, 0))
            s1 = work.tile([128, L, D], F16, tag="p1", name="s1", bufs=3)
            nc.vector.tensor_add(s1, p1, p2)
            s2 = work.tile([128, L, D // 2], F16, tag="s2", name="s2", bufs=2)
            nc.vector.tensor_add(s2, s1[:, :, 0:D // 2], s1[:, :, D // 2:D])
            s3 = work.tile([128, L, D // 4], F16, tag="s3", name="s3", bufs=2)
            nc.vector.tensor_add(s3, s2[:, :, 0:D // 4], s2[:, :, D // 4:D // 2])
            e = work.tile([128, L], F16, tag="e", name="e", bufs=2)
            nc.vector.tensor_reduce(e, s3[:], axis=mybir.AxisListType.X,
                                    op=mybir.AluOpType.add)
            m = work.tile([128, 1], F32, tag="m", name="m", bufs=2)
            nc.vector.tensor_reduce(m, e[:], axis=mybir.AxisListType.X,
                                    op=mybir.AluOpType.max)
            nm = work.tile([128, 1], F32, tag="nm", name="nm", bufs=2)
            nc.vector.tensor_scalar_mul(nm, m, -1.0 / SQD)
            al = work.tile([128, L], F32, tag="al", name="al", bufs=2)
            ssum = work.tile([128, 1], F32, tag="ssum", name="ssum", bufs=2)
            nc.scalar.activation(al, e, ACT.Exp, bias=nm, scale=1.0 / SQD,
                                 accum_out=ssum)
            rinv = work.tile([128, 1], F32, tag="rinv", name="rinv", bufs=2)
            nc.vector.reciprocal(rinv, ssum)
            al16 = work.tile([128, L], F16, tag="al16", name="al16", bufs=2)
            nc.vector.tensor_scalar_mul(al16, al, rinv)
            p3 = work.tile([128, L, D], F16, tag="p1", name="p3", bufs=3)
            nc.vector.tensor_mul(p3, rsp, _bcast(al16[:], D, 1))
            p3v = bass.AP(tensor=p3.tensor, offset=p3.offset,
                          ap=[list(p3.ap[0]), [1, D], [D, L]])
            nc.vector.tensor_reduce(rball[:, tt, :], p3v,
                                    axis=mybir.AxisListType.X,
                                    op=mybir.AluOpType.add, opt_input=False)

        TPC = NT // NCH
        for chi in range(NCH):
            emit_c_chunk(chi)
            for tt in range(TPC * chi, TPC * chi + TPC):
                emit_b_tile(tt)

    # ---------------- Phase B tail: rbar transpose + item output ----------------
    with tc.tile_pool(name="phBt_s", bufs=2) as phbts, \
         tc.tile_pool(name="phBt_ps", bufs=2, space="PSUM") as phbtps:
        for tt in range(NT):
            pst = phbtps.tile([128, 128], F16, tag="rbT")
            nc.tensor.transpose(pst, rball[:, tt, :], ident)
            rbT = phbts.tile([128, 128], F16, tag="rbTs")
            nc.vector.tensor_copy(rbT, pst)
            psI = phbtps.tile([128, 128], F32, tag="psI")
            nc.tensor.matmul(psI, w16["wl4a"], ia_own[:, tt * 128:(tt + 1) * 128],
                             start=True, stop=False)
            nc.tensor.matmul(psI, w16["wl4b"], rbT, start=False, stop=True)
            nc.scalar.activation(oi_sb[:, tt * 128:(tt + 1) * 128], psI,
                                 ACT.Identity, bias=bias["bl4"])
        nc.sync.dma_start(out=t["out_i"].ap(), in_=oi_sb)


def build_program():
    if "nc" in _CACHE:
        return _CACHE["nc"], _CACHE["names"]
    nc = bacc.Bacc("TRN2", target_bir_lowering=False, debug=False,
                   num_devices=NCORES)
    t = {}

    def din(name, shape, dtype):
        t[name] = nc.dram_tensor(name, shape, dtype, kind="ExternalInput")

    for n_ in ("wl1a", "wl1b", "wl3", "wq1", "wq3", "wl4a", "wl4b",
               "wLo2", "wfo1", "wC", "wfo2", "wLo1", "wLq",
               "wWuraw", "wAW", "wW1", "wW2"):
        din(n_, [D, D], F16)
    for n_ in ("bq1", "bLo", "bLq", "bfo", "bl1", "bl3", "bl4", "kini"):
        din(n_, [D, 1], F32)
    din("ist_own", [128, NPC], F16)
    din("skl_own", [128, NPC], F16)
    din("ufg", [NT, 128, L * D], F16)     # user_feat[item mailbox idx], natural
    din("iresp", [NT, 128, L * D], F16)
    din("rT", [L, 128, NPC], F16)         # user_mail_resp, transposed
    din("ifgT", [NCH, 128, L, UCH], F16)  # item_feat[user mailbox idx], transposed
    din("istgT", [NCH, 128, L, UCH], F16)
    din("sklgT", [NCH, 128, L, UCH], F16)
    t["out_u"] = nc.dram_tensor("out_u", [128, NPC], F16, kind="ExternalOutput")
    t["out_i"] = nc.dram_tensor("out_i", [128, NPC], F32, kind="ExternalOutput")

    with tile.TileContext(nc) as tc:
        with ExitStack() as ctx:
            _emit(ctx, tc, t)
    nc.compile()
    _CACHE["nc"] = nc
    _CACHE["names"] = t
    return nc, t


def prepare_inputs(user_feat, item_feat, user_static, item_static, skill,
                   user_mail_item_idx, user_mail_resp, item_mail_user_idx,
                   item_mail_resp, params):
    p = {k: np.asarray(v, dtype=np.float32) for k, v in params.items()}
    f16 = np.float16

    A, B = p["q1_W"][:, :D], p["q1_W"][:, D:2 * D]
    AW = A @ p["Wi"]                      # folded (A @ Wi)
    W1 = B @ p["l1_W"][:, :D]
    W2 = B @ p["l1_W"][:, D:]
    bq1f = p["q1_b"] + B @ p["l1_b"]      # folded bias

    rep = {
        "wl1a": np.ascontiguousarray(p["l1_W"][:, :D].T, dtype=f16),
        "wl1b": np.ascontiguousarray(p["l1_W"][:, D:].T, dtype=f16),
        "wl3": np.ascontiguousarray(p["l3_W"].T, dtype=f16),
        "wq1": np.ascontiguousarray(p["l2_W"][:, :D], dtype=f16),
        "wq3": np.ascontiguousarray(p["l2_W"][:, 2 * D:], dtype=f16),
        "wl4a": np.ascontiguousarray(p["l4_W"][:, :D].T, dtype=f16),
        "wl4b": np.ascontiguousarray(p["l4_W"][:, D:].T, dtype=f16),
        "wLo2": np.ascontiguousarray(p["Lo_W"][:, D:].T, dtype=f16),
        "wfo1": np.ascontiguousarray(p["fo_W"][:, :D].T, dtype=f16),
        "wC": np.ascontiguousarray(p["q1_W"][:, 2 * D:].T, dtype=f16),
        "wfo2": np.ascontiguousarray(p["fo_W"][:, D:].T, dtype=f16),
        "wLo1": np.ascontiguousarray(p["Lo_W"][:, :D].T, dtype=f16),
        "wLq": np.ascontiguousarray(p["Lq_W"].T, dtype=f16),
        "wWuraw": np.ascontiguousarray(p["Wu"], dtype=f16),
        "wAW": np.ascontiguousarray(AW.T, dtype=f16),
        "wW1": np.ascontiguousarray(W1.T, dtype=f16),
        "wW2": np.ascontiguousarray(W2.T, dtype=f16),
        "bq1": np.ascontiguousarray(bq1f[:, None], dtype=np.float32),
        "bLo": np.ascontiguousarray(p["Lo_b"][:, None], dtype=np.float32),
        "bLq": np.ascontiguousarray(p["Lq_b"][:, None], dtype=np.float32),
        "bfo": np.ascontiguousarray(p["fo_b"][:, None], dtype=np.float32),
        "bl1": np.ascontiguousarray(p["l1_b"][:, None], dtype=np.float32),
        "bl3": np.ascontiguousarray(p["l3_b"][:, None], dtype=np.float32),
        "bl4": np.ascontiguousarray(p["l4_b"][:, None], dtype=np.float32),
        "kini": np.ascontiguousarray(p["k_init"][0][:, None], dtype=np.float32),
    }

    uidx_full = np.asarray(user_mail_item_idx).astype(np.int64)
    iidx_full = np.asarray(item_mail_user_idx).astype(np.int64)
    uresp = np.asarray(user_mail_resp, dtype=np.float32)
    iresp = np.asarray(item_mail_resp, dtype=np.float32)
    uf16 = np.asarray(user_feat).astype(f16)
    if16 = np.asarray(item_feat).astype(f16)
    ist16 = np.asarray(item_static).astype(f16)
    skl16 = np.asarray(skill).astype(f16)
    ist_T = np.asarray(item_static).T
    skl_T = np.asarray(skill).T

    def stage_user_gather(tab16, idx):
        # [1024, L, D] -> [NCH, 128(d), L, UCH] (transposed, per chunk)
        g = tab16[idx]                                  # [1024, L, D]
        g = g.reshape(NCH, UCH, L, D).transpose(0, 3, 2, 1)
        return np.ascontiguousarray(g)

    in_maps = []
    for c in range(NCORES):
        s = slice(c * NPC, (c + 1) * NPC)
        ui = uidx_full[s]
        m = {
            "ist_own": np.ascontiguousarray(ist_T[:, s], dtype=f16),
            "skl_own": np.ascontiguousarray(skl_T[:, s], dtype=f16),
            "ufg": np.ascontiguousarray(
                uf16[iidx_full[s]].reshape(NT, 128, L * D)),
            "iresp": np.ascontiguousarray(
                iresp[s].reshape(NT, 128, L * D), dtype=f16),
            "rT": np.ascontiguousarray(uresp[s].transpose(1, 2, 0), dtype=f16),
            "ifgT": stage_user_gather(if16, ui),
            "istgT": stage_user_gather(ist16, ui),
            "sklgT": stage_user_gather(skl16, ui),
        }
        m.update(rep)
        in_maps.append(m)
    return in_maps


def kernel(user_feat, item_feat, user_static, item_static, skill,
           user_mail_item_idx, user_mail_resp, item_mail_user_idx,
           item_mail_resp, params):
    nc, _ = build_program()
    in_maps = prepare_inputs(
        user_feat, item_feat, user_static, item_static, skill,
        user_mail_item_idx, user_mail_resp, item_mail_user_idx,
        item_mail_resp, params)
    trace = os.environ.get("KERNEL_TRACE", "0") == "1"
    res = bass_utils.run_bass_kernel_spmd(
        nc, in_maps, core_ids=list(range(NCORES)), trace=trace)
    _CACHE["last_result"] = res
    user_new = np.concatenate(
        [res.results[c]["out_u"].T for c in range(NCORES)], axis=0)
    item_new = np.concatenate(
        [res.results[c]["out_i"].T for c in range(NCORES)], axis=0)
    return (np.ascontiguousarray(user_new, dtype=np.float32),
            np.ascontiguousarray(item_new, dtype=np.float32))
